# revision 29
# baseline (speedup 1.0000x reference)
"""DeepGCNLayer (GraphConv norm='both' + BatchNorm + ReLU + residual) on 8 trn2 cores.

Sharding: nodes padded to NPAD=100352, split into 8 ranges (98 node-tiles of
128 per core). Edges routed to the core owning their dst (dst-sorted), then
per (dst-tile, src-window) padded to a uniform K_w chunks of 128 so every
core runs one SPMD program.

v2 layout: the AllGather'd z table is built in 4 row-interleaved windows
(each window = the same quarter of every core's slice) so per-window
AllGathers overlap the z pass and the first gather groups. The one-hot S
matrices are built as ACT-engine broadcast expansion of eloc plus a DVE
is_equal on real tiles (2x perf mode, short shared-port holds) - the v1
broadcast tensor_tensor held the DVE shared SBUF port for ~28us/group,
starving the SWDGE gather descriptor generator (see trainium-docs
memories/01-sbuf.md "DVE blocks DMA" trap). x ships transposed bf16 so the
z pass needs no PE transposes; norm_src/norm_dst fold into ACT scale-copies.
"""

import sys

if "/opt/trn_rl_repo" not in sys.path:
    sys.path.insert(0, "/opt/trn_rl_repo")

import numpy as np

P = 128
D = 128
NCORES = 8
BN_EPS = 1e-5
GROUP = 7            # dst tiles per gather group
PBC = 7              # node tiles per phase-B load/store batch
WTILES = (22, 27, 27, 22)   # z-table window sizes in node tiles (per core)

_NC_CACHE = {}


def build_program(npad, nt, kws, n_real):
    """kws: tuple of chunks-per-window per dst tile (uniform across tiles)."""
    import concourse.bacc as bacc
    import concourse.tile as tile
    from concourse import mybir

    f32 = mybir.dt.float32
    bf16 = mybir.dt.bfloat16
    i32 = mybir.dt.int32
    i16 = mybir.dt.int16
    OP = mybir.AluOpType
    AF = mybir.ActivationFunctionType

    nodes_pc = nt * P
    ktot = sum(kws)
    nwin = len(kws)
    ngroups = nt // GROUP
    assert nt % GROUP == 0
    assert sum(WTILES) == nt and len(WTILES) == nwin
    kmax = max(kws)
    # idx16 columns per (group, window); eloc columns per (group, window)
    cols_gw = [GROUP * kw * P // 16 for kw in kws]
    gcols_i = sum(cols_gw)              # idx cols per group
    gcols_e = GROUP * ktot              # eloc cols per group

    nc = bacc.Bacc("TRN2", target_bir_lowering=False, debug=False,
                   num_devices=NCORES, num_swdge_queues=4)

    xt = nc.dram_tensor("xt", [P, nodes_pc], bf16, kind="ExternalInput")
    xres = nc.dram_tensor("xres", [nodes_pc, D], f32, kind="ExternalInput")
    wmat = nc.dram_tensor("wmat", [D, D], bf16, kind="ExternalInput")
    grow = nc.dram_tensor("grow", [1, D], f32, kind="ExternalInput")
    brow = nc.dram_tensor("brow", [1, D], f32, kind="ExternalInput")
    dgo = nc.dram_tensor("dgo", [P, nt], i32, kind="ExternalInput")
    dgi = nc.dram_tensor("dgi", [P, nt], i32, kind="ExternalInput")
    idxs = nc.dram_tensor("idxs", [P, ngroups * gcols_i], i16,
                          kind="ExternalInput")
    eloc = nc.dram_tensor("eloc", [P, ngroups * gcols_e], bf16,
                          kind="ExternalInput")
    out = nc.dram_tensor("out", [nodes_pc, D], f32, kind="ExternalOutput")
    h_d = nc.dram_tensor("h_d", [nodes_pc, D], bf16)

    # per-window z contribution + AllGather'd table (row-interleaved:
    # window w = [core0 quarter w | core1 quarter w | ...])
    cc_z = [nc.dram_tensor(f"cc_z{w}", [WTILES[w] * P, D], bf16)
            for w in range(nwin)]
    z_t = [nc.dram_tensor(f"z_t{w}", [WTILES[w] * P * NCORES, D], bf16,
                          addr_space="Shared")
           for w in range(nwin)]

    with tile.TileContext(nc) as tc:
        with (
            tc.tile_pool(name="const", bufs=1) as constp,
            tc.tile_pool(name="norm", bufs=1) as normp,
            tc.tile_pool(name="xz", bufs=2) as xzp,
            tc.tile_pool(name="zst", bufs=2) as zsp,
            tc.tile_pool(name="meta", bufs=2) as metap,
            tc.tile_pool(name="gathA", bufs=3) as gathA,
            tc.tile_pool(name="gathB", bufs=3) as gathB,
            tc.tile_pool(name="s", bufs=1) as sp,
            tc.tile_pool(name="work", bufs=2) as workp,
            tc.tile_pool(name="stats", bufs=1) as statp,
            tc.tile_pool(name="io", bufs=3) as iop,
            tc.tile_pool(name="psA", bufs=2, space="PSUM") as psA,
            tc.tile_pool(name="psB", bufs=2, space="PSUM") as psB,
            tc.tile_pool(name="psS", bufs=1, space="PSUM") as psS,
            tc.tile_pool(name="dram", bufs=2, space="DRAM") as dramp,
        ):
            # ---- constants -------------------------------------------------
            iota = constp.tile([P, P], bf16, tag="iota")
            nc.gpsimd.iota(iota[:], pattern=[[1, P]], base=0,
                           channel_multiplier=0,
                           allow_small_or_imprecise_dtypes=True)
            # iota replicated along free dim for batched 2x is_equal
            iota_rep = constp.tile([P, GROUP * kmax * P], bf16, tag="iotar")
            nc.vector.tensor_copy(
                iota_rep[:].rearrange("p (c e) -> p c e", e=P),
                iota[:, None, :].to_broadcast([P, GROUP * kmax, P]))
            ones1 = constp.tile([1, P], f32, tag="ones1")
            nc.vector.memset(ones1[:], 1.0)
            ones_c = constp.tile([P, 1], bf16, tag="ones_c")
            nc.vector.memset(ones_c[:], 1.0)
            w_sb = constp.tile([P, D], bf16, tag="wsb")
            nc.sync.dma_start(out=w_sb[:], in_=wmat[:])
            g_row = constp.tile([1, D], f32, tag="grow")
            nc.sync.dma_start(out=g_row[:], in_=grow[:])
            be_row = constp.tile([1, D], f32, tag="berow")
            nc.sync.dma_start(out=be_row[:], in_=brow[:])

            # ---- norm arrays (own range, F-order [P, nt]) ------------------
            deg = normp.tile([P, nt], i32, tag="deg")
            nc.sync.dma_start(out=deg[:], in_=dgo[:])
            degf = normp.tile([P, nt], f32, tag="degf")
            nc.vector.tensor_scalar_max(degf[:], deg[:], 1.0)
            nc.scalar.sqrt(degf[:], degf[:])
            ns_f = constp.tile([P, nt], f32, tag="ns_f")
            nc.vector.reciprocal(ns_f[:], degf[:])

            deg2 = normp.tile([P, nt], i32, tag="deg2")
            nc.sync.dma_start(out=deg2[:], in_=dgi[:])
            deg2f = normp.tile([P, nt], f32, tag="deg2f")
            nc.vector.tensor_scalar_max(deg2f[:], deg2[:], 1.0)
            nc.scalar.sqrt(deg2f[:], deg2f[:])
            nd_f = constp.tile([P, nt], f32, tag="nd_f")
            nc.vector.reciprocal(nd_f[:], deg2f[:])

            # ---- z pass: z = (x@W)*ns -> bf16, own slice, 4 windows --------
            # lhsT = xT slice (no transposes); ns folds into the ACT
            # PSUM->SBUF copy; each window's AllGather issues as soon as its
            # quarter is stored so window-0 gathers start early.
            t0 = 0
            for w in range(nwin):
                wt = WTILES[w]
                xt_w = xzp.tile([P, wt * P], bf16, tag="xt_w")
                nc.sync.dma_start(out=xt_w[:],
                                  in_=xt[:, t0 * P:(t0 + wt) * P])
                z_w = zsp.tile([P, wt * D], bf16, tag="z_w")
                for c in range(wt):
                    z_ps = psB.tile([P, D], f32, tag="B")
                    nc.tensor.matmul(out=z_ps[:],
                                     lhsT=xt_w[:, c * P:(c + 1) * P],
                                     rhs=w_sb[:], start=True, stop=True)
                    nc.scalar.activation(
                        out=z_w[:, c * D:(c + 1) * D], in_=z_ps[:],
                        func=AF.Copy, scale=ns_f[:, t0 + c:t0 + c + 1])
                nc.scalar.dma_start(
                    out=cc_z[w][:].rearrange("(c p) e -> p c e", p=P),
                    in_=z_w[:].rearrange("p (c e) -> p c e", e=D))
                nc.gpsimd.collective_compute(
                    "AllGather", OP.bypass,
                    replica_groups=[list(range(NCORES))],
                    ins=[cc_z[w][:]], outs=[z_t[w][:]])
                t0 += wt

            # ---- phase A ---------------------------------------------------
            sum_ps = psS.tile([1, P], f32, tag="sum")
            sq_ps = psS.tile([1, P], f32, tag="sq")

            for g in range(ngroups):
                idx_g = metap.tile([P, gcols_i], i16, tag="idxg")
                nc.sync.dma_start(
                    out=idx_g[:], in_=idxs[:, g * gcols_i:(g + 1) * gcols_i])
                eloc_g = metap.tile([P, gcols_e], bf16, tag="elocg")
                nc.sync.dma_start(
                    out=eloc_g[:], in_=eloc[:, g * gcols_e:(g + 1) * gcols_e])

                e_ws = []
                s_ws = []
                ico = 0
                eco = 0
                for w in range(nwin):
                    kw = kws[w]
                    nch = GROUP * kw
                    nidx = nch * P
                    pool = gathA if w < 2 else gathB
                    e_t = pool.tile([P, nch * D], bf16, tag=f"E{w}")
                    nc.gpsimd.dma_gather(
                        e_t[:].rearrange("p (c e) -> p c e", e=D),
                        z_t[w][:],
                        idx_g[:, ico:ico + nidx // 16],
                        nidx, nidx, D, single_packet=False,
                        queue_num=(g + w) % 4)
                    ico += nidx // 16
                    e_ws.append(e_t)
                    s_t = sp.tile([P, nch * P], bf16, tag=f"S{w}")
                    nc.vector.tensor_tensor(
                        out=s_t[:].rearrange("p (c e) -> p c e", e=P),
                        in0=eloc_g[:, eco:eco + nch, None].to_broadcast(
                            [P, nch, P]),
                        in1=iota_rep[:, :nch * P].rearrange(
                            "p (c e) -> p c e", e=P),
                        op=OP.is_equal)
                    eco += nch
                    s_ws.append(s_t)

                h_g = workp.tile([P, GROUP * D], bf16, tag="hg")
                for u in range(GROUP):
                    t = g * GROUP + u
                    agg_ps = psA.tile([P, P], f32, tag="A")
                    ci = 0
                    for w in range(nwin):
                        kw = kws[w]
                        for j in range(kw):
                            nc.tensor.matmul(
                                out=agg_ps[:],
                                lhsT=s_ws[w][:, (u * kw + j) * P:
                                             (u * kw + j + 1) * P],
                                rhs=e_ws[w][:, (u * kw + j) * D:
                                            (u * kw + j + 1) * D],
                                start=(ci + j == 0),
                                stop=(ci + j == ktot - 1))
                        ci += kw
                    h_t = h_g[:, u * D:(u + 1) * D]
                    nc.scalar.activation(out=h_t, in_=agg_ps[:],
                                         func=AF.Copy,
                                         scale=nd_f[:, t:t + 1])
                    sq_sb = workp.tile([P, D], bf16, tag="sqsb")
                    nc.scalar.activation(out=sq_sb[:], in_=h_t,
                                         func=AF.Square)
                    nc.tensor.matmul(out=sum_ps[:], lhsT=ones_c[:], rhs=h_t,
                                     start=(t == 0), stop=(t == nt - 1))
                    nc.tensor.matmul(out=sq_ps[:], lhsT=ones_c[:],
                                     rhs=sq_sb[:],
                                     start=(t == 0), stop=(t == nt - 1))
                nc.scalar.dma_start(
                    out=h_d[:].rearrange("(c p) e -> p c e", p=P)[
                        :, g * GROUP:(g + 1) * GROUP, :],
                    in_=h_g[:].rearrange("p (c e) -> p c e", e=D))

            # ---- BN stats all-reduce + scale/shift (row layout) ------------
            srow = statp.tile([1, 2 * P], f32, tag="srow")
            nc.scalar.copy(srow[0:1, 0:P], sum_ps[:])
            nc.scalar.copy(srow[0:1, P:2 * P], sq_ps[:])
            cc_in = dramp.tile([1, 2 * P], f32, tag="ccin")
            cc_out = dramp.tile([1, 2 * P], f32, tag="ccout")
            nc.gpsimd.dma_start(out=cc_in[:], in_=srow[:])
            nc.gpsimd.collective_compute(
                "AllReduce", OP.add,
                replica_groups=[list(range(NCORES))],
                ins=[cc_in.opt()], outs=[cc_out.opt()])
            grow_sb = statp.tile([1, 2 * P], f32, tag="grow_sb")
            nc.gpsimd.dma_start(out=grow_sb[:], in_=cc_out[:])

            inv_n = 1.0 / float(n_real)
            mean_r = statp.tile([1, P], f32, tag="mean")
            nc.vector.tensor_scalar_mul(mean_r[:], grow_sb[0:1, 0:P], inv_n)
            ex2_r = statp.tile([1, P], f32, tag="ex2")
            nc.vector.tensor_scalar_mul(ex2_r[:], grow_sb[0:1, P:2 * P],
                                        inv_n)
            m2_r = statp.tile([1, P], f32, tag="m2")
            nc.scalar.square(m2_r[:], mean_r[:])
            var_r = statp.tile([1, P], f32, tag="var")
            nc.vector.tensor_tensor(out=var_r[:], in0=ex2_r[:], in1=m2_r[:],
                                    op=OP.subtract)
            nc.vector.tensor_scalar_add(var_r[:], var_r[:], BN_EPS)
            sd_r = statp.tile([1, P], f32, tag="sd")
            nc.scalar.sqrt(sd_r[:], var_r[:])
            inv_r = statp.tile([1, P], f32, tag="inv")
            nc.vector.reciprocal(inv_r[:], sd_r[:])
            sc_r = statp.tile([1, P], f32, tag="sc")
            nc.vector.tensor_tensor(out=sc_r[:], in0=g_row[:], in1=inv_r[:],
                                    op=OP.mult)
            # b cancels in BN: shift = beta - mean*scale
            tc_r = statp.tile([1, P], f32, tag="tc")
            nc.vector.tensor_tensor(out=tc_r[:], in0=mean_r[:], in1=sc_r[:],
                                    op=OP.mult)
            nc.vector.tensor_tensor(out=tc_r[:], in0=be_row[:], in1=tc_r[:],
                                    op=OP.subtract)

            # rank-1 broadcast of sc/tc across partitions -> bf16 tiles
            scb_ps = psA.tile([P, P], f32, tag="A")
            nc.tensor.matmul(out=scb_ps[:], lhsT=ones1[:], rhs=sc_r[:],
                             start=True, stop=True)
            sc_bc = constp.tile([P, P], bf16, tag="sc_bc")
            nc.scalar.copy(sc_bc[:], scb_ps[:])
            tcb_ps = psB.tile([P, P], f32, tag="B")
            nc.tensor.matmul(out=tcb_ps[:], lhsT=ones1[:], rhs=tc_r[:],
                             start=True, stop=True)
            tc_bc = constp.tile([P, P], bf16, tag="tc_bc")
            nc.scalar.copy(tc_bc[:], tcb_ps[:])

            # ---- phase B (batched loads/stores, no transposes) -------------
            for bt in range(nt // PBC):
                t0 = bt * PBC
                x_b = iop.tile([P, PBC * D], f32, tag="xb")
                nc.scalar.dma_start(
                    out=x_b[:].rearrange("p (c e) -> p c e", e=D),
                    in_=xres[:].rearrange("(c p) e -> p c e", p=P)[
                        :, t0:t0 + PBC, :])
                h_b = iop.tile([P, PBC * D], bf16, tag="hb")
                nc.sync.dma_start(
                    out=h_b[:].rearrange("p (c e) -> p c e", e=D),
                    in_=h_d[:].rearrange("(c p) e -> p c e", p=P)[
                        :, t0:t0 + PBC, :])
                eng = nc.vector if bt % 2 == 0 else nc.gpsimd
                g1 = workp.tile([P, PBC * D], bf16, tag="g1")
                eng.tensor_tensor(
                    out=g1[:].rearrange("p (c e) -> p c e", e=D),
                    in0=h_b[:].rearrange("p (c e) -> p c e", e=D),
                    in1=sc_bc[:, None, :].to_broadcast([P, PBC, D]),
                    op=OP.mult)
                eng.tensor_tensor(
                    out=g1[:].rearrange("p (c e) -> p c e", e=D),
                    in0=g1[:].rearrange("p (c e) -> p c e", e=D),
                    in1=tc_bc[:, None, :].to_broadcast([P, PBC, D]),
                    op=OP.add)
                eng.tensor_scalar_max(g1[:], g1[:], 0.0)
                eng.tensor_tensor(out=x_b[:], in0=g1[:], in1=x_b[:],
                                  op=OP.add)
                nc.sync.dma_start(
                    out=out[:].rearrange("(c p) e -> p c e", p=P)[
                        :, t0:t0 + PBC, :],
                    in_=x_b[:].rearrange("p (c e) -> p c e", e=D))

    nc.compile()
    return nc


def _wrap16(a):
    b = a.reshape(-1, 16).T
    return np.tile(b, (8, 1))


def host_prep(x, src, dst, W, b, gamma, beta):
    """Graph routing / layout prep (indices only - no FLOPs on host)."""
    import ml_dtypes

    x = np.asarray(x, np.float32)
    W = np.asarray(W, np.float32)
    gamma = np.asarray(gamma, np.float32)
    beta = np.asarray(beta, np.float32)
    src32 = np.asarray(src).astype(np.int64)
    dst32 = np.asarray(dst).astype(np.int64)

    n = x.shape[0]
    npad = -(-n // (P * NCORES * GROUP)) * (P * NCORES * GROUP)
    nodes_pc = npad // NCORES
    nt = nodes_pc // P
    nt_tot = npad // P
    assert sum(WTILES) == nt
    nwin = len(WTILES)
    wt_start = np.cumsum([0] + list(WTILES))  # in tiles, per core

    order = np.argsort(dst32, kind="stable")
    ds = dst32[order]
    ss = src32[order]

    ar = np.arange(npad + 1, dtype=np.int64)
    rps = np.searchsorted(np.sort(src32), ar).astype(np.int32)
    rpd_full = np.searchsorted(ds, ar)

    # src -> (window, row within window table). Window w of the z table is
    # [core0 quarter w | core1 quarter w | ...], quarter w = tiles
    # [wt_start[w], wt_start[w+1]) of each core's slice.
    s_core = ss // nodes_pc
    s_r = ss % nodes_pc
    s_tile = s_r // P
    s_win = np.searchsorted(wt_start, s_tile, side="right") - 1
    wrows = (np.array(WTILES) * P)[s_win]
    s_winrow = s_core * wrows + (s_r - wt_start[s_win] * P)

    # degree counts (int), F-order [P, nt] per core
    dgo_n = np.diff(rps).astype(np.int32)                 # [npad]
    dgi_n = np.diff(rpd_full).astype(np.int32)            # [npad]

    # per-dst in-degree split by src window
    deg4 = np.zeros((npad, nwin), np.int32)
    np.add.at(deg4, (ds, s_win), 1)

    # --- bin-pack dst nodes into tiles (per core) to flatten the
    # per-(tile, window) edge-count tails, so kws (chunk counts) shrink.
    caps = np.array([4 * P, 5 * P, 5 * P, 4 * P], np.float64)
    newpos = np.empty(npad, np.int64)    # global node -> permuted local slot
    for c in range(NCORES):
        d4 = deg4[c * nodes_pc:(c + 1) * nodes_pc].astype(np.float64)
        order_c = np.argsort(-d4.sum(1), kind="stable")
        loads = np.zeros((nt, nwin))
        counts = np.zeros(nt, np.int64)
        fill = [[] for _ in range(nt)]
        for i in order_c:
            util = np.max((loads + d4[i]) / caps, axis=1)
            util[counts >= P] = np.inf
            b = int(np.argmin(util))
            loads[b] += d4[i]
            counts[b] += 1
            fill[b].append(i)
        pos = np.empty(nodes_pc, np.int64)
        for b in range(nt):
            pos[np.array(fill[b], np.int64)] = (
                b * P + np.arange(len(fill[b])))
        newpos[c * nodes_pc:(c + 1) * nodes_pc] = pos

    # kws from the packed loads (global max over cores/tiles per window)
    e_core = ds // nodes_pc
    e_pos = newpos[ds]
    e_tile = e_core * nt + e_pos // P
    e_slot = e_pos % P
    cell = e_tile * nwin + s_win
    cnt = np.bincount(cell, minlength=nt_tot * nwin).reshape(nt_tot, nwin)
    kws = tuple(int(-(-cnt[:, w].max() // P)) for w in range(nwin))
    ktot = sum(kws)

    # per (tile, window) edge lists under the permutation
    eorder = np.argsort(cell, kind="stable")
    bnd = np.searchsorted(cell[eorder], np.arange(nt_tot * nwin + 1))
    tw_lists = [[eorder[bnd[t * nwin + w]:bnd[t * nwin + w + 1]]
                 for w in range(nwin)] for t in range(nt_tot)]

    xpad = np.zeros((npad, D), np.float32)
    xpad[:n] = x

    ngroups = nt // GROUP
    in_maps = []
    shared = dict(
        wmat=W.astype(ml_dtypes.bfloat16),
        grow=np.ascontiguousarray(gamma[None, :]),
        brow=np.ascontiguousarray(beta[None, :]))
    for c in range(NCORES):
        # eloc layout: (g, w, u, chunk) contiguous for batched expansions
        elocv = np.full((nt * ktot, P), -1.0, np.float32)
        idx_blocks = []
        ecol_off = 0
        for g in range(ngroups):
            for w in range(nwin):
                blk = np.zeros(GROUP * kws[w] * P, np.int16)
                for u in range(GROUP):
                    t = g * GROUP + u
                    gt = c * nt + t
                    sel = tw_lists[gt][w]
                    base = u * kws[w] * P
                    blk[base:base + len(sel)] = s_winrow[sel].astype(np.int16)
                    ev = e_slot[sel].astype(np.float32)
                    ecol = elocv[ecol_off + u * kws[w]:
                                 ecol_off + (u + 1) * kws[w]].reshape(-1)
                    ecol[:len(sel)] = ev
                ecol_off += GROUP * kws[w]
                idx_blocks.append(_wrap16(blk))
        m = dict(shared)
        xslice = xpad[c * nodes_pc:(c + 1) * nodes_pc]
        pos_c = newpos[c * nodes_pc:(c + 1) * nodes_pc]
        orig_of = np.empty(nodes_pc, np.int64)
        orig_of[pos_c] = np.arange(nodes_pc)
        m["xt"] = np.ascontiguousarray(xslice.T).astype(ml_dtypes.bfloat16)
        m["xres"] = np.ascontiguousarray(xslice[orig_of])
        m["dgo"] = np.ascontiguousarray(
            dgo_n[c * nodes_pc:(c + 1) * nodes_pc].reshape(nt, P).T)
        m["dgi"] = np.ascontiguousarray(
            dgi_n[c * nodes_pc:(c + 1) * nodes_pc][orig_of]
            .reshape(nt, P).T)
        m["idxs"] = np.ascontiguousarray(np.concatenate(idx_blocks, axis=1))
        # eloc device layout: col (g,w,u,chunk) partition p = edge c*128+p
        m["eloc"] = np.ascontiguousarray(
            elocv.T).astype(ml_dtypes.bfloat16)
        in_maps.append(m)
    return dict(npad=npad, nt=nt, kws=kws, n_real=n, newpos=newpos), in_maps


def run(in_maps, cfg, **kw):
    from concourse.bass_utils import run_bass_kernel_spmd

    key = (cfg["npad"], cfg["nt"], tuple(cfg["kws"]), cfg["n_real"])
    if key not in _NC_CACHE:
        _NC_CACHE[key] = build_program(*key)
    nc = _NC_CACHE[key]
    res = run_bass_kernel_spmd(nc, in_maps, core_ids=list(range(NCORES)), **kw)
    n = cfg["n_real"]
    nodes_pc = cfg["npad"] // NCORES
    parts = []
    for c in range(NCORES):
        o = np.asarray(res.results[c]["out"])
        pos_c = cfg["newpos"][c * nodes_pc:(c + 1) * nodes_pc]
        parts.append(o[pos_c])
    full = np.concatenate(parts, axis=0)[:n]
    return np.ascontiguousarray(full, dtype=np.float32), res


def kernel(x, src, dst, W, b, gamma, beta):
    cfg, in_maps = host_prep(x, src, dst, W, b, gamma, beta)
    out, _ = run(in_maps, cfg)
    return out


# revision 35
# speedup vs baseline: 1.0423x; 1.0423x over previous
"""DeepGCNLayer (GraphConv norm='both' + BatchNorm + ReLU + residual) on 8 trn2 cores.

Sharding: nodes padded to NPAD=100352, split into 8 ranges (98 node-tiles of
128 per core). Edges routed to the core owning their dst (dst-sorted), then
per (dst-tile, src-window) padded to a uniform K_w chunks of 128 so every
core runs one SPMD program.

v2 layout: the AllGather'd z table is built in 4 row-interleaved windows
(each window = the same quarter of every core's slice) so per-window
AllGathers overlap the z pass and the first gather groups. The one-hot S
matrices are built as ACT-engine broadcast expansion of eloc plus a DVE
is_equal on real tiles (2x perf mode, short shared-port holds) - the v1
broadcast tensor_tensor held the DVE shared SBUF port for ~28us/group,
starving the SWDGE gather descriptor generator (see trainium-docs
memories/01-sbuf.md "DVE blocks DMA" trap). x ships transposed bf16 so the
z pass needs no PE transposes; norm_src/norm_dst fold into ACT scale-copies.
"""

import sys

if "/opt/trn_rl_repo" not in sys.path:
    sys.path.insert(0, "/opt/trn_rl_repo")

import numpy as np

P = 128
D = 128
NCORES = 8
BN_EPS = 1e-5
GROUP = 7            # dst tiles per gather group
PBC = 7              # node tiles per phase-B load/store batch
WTILES = (14, 28, 28, 28)   # z-table window sizes in node tiles (per core)

_NC_CACHE = {}


def build_program(npad, nt, kws, n_real):
    """kws: tuple of chunks-per-window per dst tile (uniform across tiles)."""
    import concourse.bacc as bacc
    import concourse.tile as tile
    from concourse import mybir

    f32 = mybir.dt.float32
    bf16 = mybir.dt.bfloat16
    i32 = mybir.dt.int32
    i16 = mybir.dt.int16
    OP = mybir.AluOpType
    AF = mybir.ActivationFunctionType

    nodes_pc = nt * P
    ktot = sum(kws)
    nwin = len(kws)
    ngroups = nt // GROUP
    assert nt % GROUP == 0
    assert sum(WTILES) == nt and len(WTILES) == nwin
    kmax = max(kws)
    # idx16 columns per (group, window); eloc columns per (group, window)
    cols_gw = [GROUP * kw * P // 16 for kw in kws]
    gcols_i = sum(cols_gw)              # idx cols per group
    gcols_e = GROUP * ktot              # eloc cols per group

    nc = bacc.Bacc("TRN2", target_bir_lowering=False, debug=False,
                   num_devices=NCORES, num_swdge_queues=4)

    xt = nc.dram_tensor("xt", [P, nodes_pc], bf16, kind="ExternalInput")
    xres = nc.dram_tensor("xres", [nodes_pc, D], f32, kind="ExternalInput")
    wmat = nc.dram_tensor("wmat", [D, D], bf16, kind="ExternalInput")
    grow = nc.dram_tensor("grow", [1, D], f32, kind="ExternalInput")
    brow = nc.dram_tensor("brow", [1, D], f32, kind="ExternalInput")
    dgo = nc.dram_tensor("dgo", [P, nt], i32, kind="ExternalInput")
    dgi = nc.dram_tensor("dgi", [P, nt], i32, kind="ExternalInput")
    idxs = nc.dram_tensor("idxs", [P, ngroups * gcols_i], i16,
                          kind="ExternalInput")
    eloc = nc.dram_tensor("eloc", [P, ngroups * gcols_e], bf16,
                          kind="ExternalInput")
    out = nc.dram_tensor("out", [nodes_pc, D], f32, kind="ExternalOutput")
    h_d = nc.dram_tensor("h_d", [nodes_pc, D], bf16)

    # per-window z contribution + AllGather'd table (row-interleaved:
    # window w = [core0 quarter w | core1 quarter w | ...])
    cc_z = [nc.dram_tensor(f"cc_z{w}", [WTILES[w] * P, D], bf16)
            for w in range(nwin)]
    z_t = [nc.dram_tensor(f"z_t{w}", [WTILES[w] * P * NCORES, D], bf16,
                          addr_space="Shared")
           for w in range(nwin)]

    with tile.TileContext(nc) as tc:
        with (
            tc.tile_pool(name="const", bufs=1) as constp,
            tc.tile_pool(name="norm", bufs=1) as normp,
            tc.tile_pool(name="xz", bufs=1) as xzp,
            tc.tile_pool(name="zst", bufs=1) as zsp,
            tc.tile_pool(name="meta", bufs=2) as metap,
            tc.tile_pool(name="gathA", bufs=3) as gathA,
            tc.tile_pool(name="gathB", bufs=3) as gathB,
            tc.tile_pool(name="s", bufs=1) as sp,
            tc.tile_pool(name="work", bufs=2) as workp,
            tc.tile_pool(name="stats", bufs=1) as statp,
            tc.tile_pool(name="io", bufs=3) as iop,
            tc.tile_pool(name="psA", bufs=2, space="PSUM") as psA,
            tc.tile_pool(name="psB", bufs=2, space="PSUM") as psB,
            tc.tile_pool(name="psS", bufs=1, space="PSUM") as psS,
            tc.tile_pool(name="dram", bufs=2, space="DRAM") as dramp,
        ):
            # ---- constants -------------------------------------------------
            iota = constp.tile([P, P], bf16, tag="iota")
            nc.gpsimd.iota(iota[:], pattern=[[1, P]], base=0,
                           channel_multiplier=0,
                           allow_small_or_imprecise_dtypes=True)
            # iota replicated along free dim for batched 2x is_equal
            iota_rep = constp.tile([P, GROUP * kmax * P], bf16, tag="iotar")
            nc.vector.tensor_copy(
                iota_rep[:].rearrange("p (c e) -> p c e", e=P),
                iota[:, None, :].to_broadcast([P, GROUP * kmax, P]))
            ones1 = constp.tile([1, P], f32, tag="ones1")
            nc.vector.memset(ones1[:], 1.0)
            ones_c = constp.tile([P, 1], bf16, tag="ones_c")
            nc.vector.memset(ones_c[:], 1.0)
            w_sb = constp.tile([P, D], bf16, tag="wsb")
            nc.sync.dma_start(out=w_sb[:], in_=wmat[:])
            g_row = constp.tile([1, D], f32, tag="grow")
            nc.sync.dma_start(out=g_row[:], in_=grow[:])
            be_row = constp.tile([1, D], f32, tag="berow")
            nc.sync.dma_start(out=be_row[:], in_=brow[:])

            # ---- norm arrays (own range, F-order [P, nt]) ------------------
            deg = normp.tile([P, nt], i32, tag="deg")
            nc.sync.dma_start(out=deg[:], in_=dgo[:])
            degf = normp.tile([P, nt], f32, tag="degf")
            nc.vector.tensor_scalar_max(degf[:], deg[:], 1.0)
            nc.scalar.sqrt(degf[:], degf[:])
            ns_f = constp.tile([P, nt], f32, tag="ns_f")
            nc.vector.reciprocal(ns_f[:], degf[:])

            deg2 = normp.tile([P, nt], i32, tag="deg2")
            nc.sync.dma_start(out=deg2[:], in_=dgi[:])
            deg2f = normp.tile([P, nt], f32, tag="deg2f")
            nc.vector.tensor_scalar_max(deg2f[:], deg2[:], 1.0)
            nc.scalar.sqrt(deg2f[:], deg2f[:])
            nd_f = constp.tile([P, nt], f32, tag="nd_f")
            nc.vector.reciprocal(nd_f[:], deg2f[:])

            # ---- z pass: z = (x@W)*ns -> bf16, own slice, 4 windows --------
            # lhsT = xT slice (no transposes); ns folds into the ACT
            # PSUM->SBUF copy; each window's AllGather issues as soon as its
            # quarter is stored so window-0 gathers start early.
            t0 = 0
            for w in range(nwin):
                wt = WTILES[w]
                xt_w = xzp.tile([P, wt * P], bf16, tag="xt_w")
                nc.sync.dma_start(out=xt_w[:],
                                  in_=xt[:, t0 * P:(t0 + wt) * P])
                z_w = zsp.tile([P, wt * D], bf16, tag="z_w")
                for c in range(wt):
                    z_ps = psB.tile([P, D], f32, tag="B")
                    nc.tensor.matmul(out=z_ps[:],
                                     lhsT=xt_w[:, c * P:(c + 1) * P],
                                     rhs=w_sb[:], start=True, stop=True)
                    nc.scalar.activation(
                        out=z_w[:, c * D:(c + 1) * D], in_=z_ps[:],
                        func=AF.Copy, scale=ns_f[:, t0 + c:t0 + c + 1])
                nc.scalar.dma_start(
                    out=cc_z[w][:].rearrange("(c p) e -> p c e", p=P),
                    in_=z_w[:].rearrange("p (c e) -> p c e", e=D))
                nc.gpsimd.collective_compute(
                    "AllGather", OP.bypass,
                    replica_groups=[list(range(NCORES))],
                    ins=[cc_z[w][:]], outs=[z_t[w][:]])
                t0 += wt

            # ---- phase A ---------------------------------------------------
            sum_ps = psS.tile([1, P], f32, tag="sum")
            sq_ps = psS.tile([1, P], f32, tag="sq")

            for g in range(ngroups):
                idx_g = metap.tile([P, gcols_i], i16, tag="idxg")
                nc.sync.dma_start(
                    out=idx_g[:], in_=idxs[:, g * gcols_i:(g + 1) * gcols_i])
                eloc_g = metap.tile([P, gcols_e], bf16, tag="elocg")
                nc.sync.dma_start(
                    out=eloc_g[:], in_=eloc[:, g * gcols_e:(g + 1) * gcols_e])

                e_ws = []
                s_ws = []
                ico = 0
                eco = 0
                for w in range(nwin):
                    kw = kws[w]
                    nch = GROUP * kw
                    nidx = nch * P
                    pool = gathA if w < 2 else gathB
                    e_t = pool.tile([P, nch * D], bf16, tag=f"E{w}")
                    nc.gpsimd.dma_gather(
                        e_t[:].rearrange("p (c e) -> p c e", e=D),
                        z_t[w][:],
                        idx_g[:, ico:ico + nidx // 16],
                        nidx, nidx, D, single_packet=False,
                        queue_num=(g + w) % 4)
                    ico += nidx // 16
                    e_ws.append(e_t)
                    s_t = sp.tile([P, nch * P], bf16, tag=f"S{w}")
                    nc.vector.tensor_tensor(
                        out=s_t[:].rearrange("p (c e) -> p c e", e=P),
                        in0=eloc_g[:, eco:eco + nch, None].to_broadcast(
                            [P, nch, P]),
                        in1=iota_rep[:, :nch * P].rearrange(
                            "p (c e) -> p c e", e=P),
                        op=OP.is_equal)
                    eco += nch
                    s_ws.append(s_t)

                h_g = workp.tile([P, GROUP * D], bf16, tag="hg")
                for u in range(GROUP):
                    t = g * GROUP + u
                    agg_ps = psA.tile([P, P], f32, tag="A")
                    ci = 0
                    for w in range(nwin):
                        kw = kws[w]
                        for j in range(kw):
                            nc.tensor.matmul(
                                out=agg_ps[:],
                                lhsT=s_ws[w][:, (u * kw + j) * P:
                                             (u * kw + j + 1) * P],
                                rhs=e_ws[w][:, (u * kw + j) * D:
                                            (u * kw + j + 1) * D],
                                start=(ci + j == 0),
                                stop=(ci + j == ktot - 1))
                        ci += kw
                    h_t = h_g[:, u * D:(u + 1) * D]
                    nc.scalar.activation(out=h_t, in_=agg_ps[:],
                                         func=AF.Copy,
                                         scale=nd_f[:, t:t + 1])
                    sq_sb = workp.tile([P, D], bf16, tag="sqsb")
                    nc.scalar.activation(out=sq_sb[:], in_=h_t,
                                         func=AF.Square)
                    nc.tensor.matmul(out=sum_ps[:], lhsT=ones_c[:], rhs=h_t,
                                     start=(t == 0), stop=(t == nt - 1))
                    nc.tensor.matmul(out=sq_ps[:], lhsT=ones_c[:],
                                     rhs=sq_sb[:],
                                     start=(t == 0), stop=(t == nt - 1))
                nc.scalar.dma_start(
                    out=h_d[:].rearrange("(c p) e -> p c e", p=P)[
                        :, g * GROUP:(g + 1) * GROUP, :],
                    in_=h_g[:].rearrange("p (c e) -> p c e", e=D))

            # ---- BN stats all-reduce + scale/shift (row layout) ------------
            srow = statp.tile([1, 2 * P], f32, tag="srow")
            nc.scalar.copy(srow[0:1, 0:P], sum_ps[:])
            nc.scalar.copy(srow[0:1, P:2 * P], sq_ps[:])
            cc_in = dramp.tile([1, 2 * P], f32, tag="ccin")
            cc_out = dramp.tile([1, 2 * P], f32, tag="ccout")
            nc.gpsimd.dma_start(out=cc_in[:], in_=srow[:])
            nc.gpsimd.collective_compute(
                "AllReduce", OP.add,
                replica_groups=[list(range(NCORES))],
                ins=[cc_in.opt()], outs=[cc_out.opt()])
            grow_sb = statp.tile([1, 2 * P], f32, tag="grow_sb")
            nc.gpsimd.dma_start(out=grow_sb[:], in_=cc_out[:])

            inv_n = 1.0 / float(n_real)
            mean_r = statp.tile([1, P], f32, tag="mean")
            nc.vector.tensor_scalar_mul(mean_r[:], grow_sb[0:1, 0:P], inv_n)
            ex2_r = statp.tile([1, P], f32, tag="ex2")
            nc.vector.tensor_scalar_mul(ex2_r[:], grow_sb[0:1, P:2 * P],
                                        inv_n)
            m2_r = statp.tile([1, P], f32, tag="m2")
            nc.scalar.square(m2_r[:], mean_r[:])
            var_r = statp.tile([1, P], f32, tag="var")
            nc.vector.tensor_tensor(out=var_r[:], in0=ex2_r[:], in1=m2_r[:],
                                    op=OP.subtract)
            nc.vector.tensor_scalar_add(var_r[:], var_r[:], BN_EPS)
            sd_r = statp.tile([1, P], f32, tag="sd")
            nc.scalar.sqrt(sd_r[:], var_r[:])
            inv_r = statp.tile([1, P], f32, tag="inv")
            nc.vector.reciprocal(inv_r[:], sd_r[:])
            sc_r = statp.tile([1, P], f32, tag="sc")
            nc.vector.tensor_tensor(out=sc_r[:], in0=g_row[:], in1=inv_r[:],
                                    op=OP.mult)
            # b cancels in BN: shift = beta - mean*scale
            tc_r = statp.tile([1, P], f32, tag="tc")
            nc.vector.tensor_tensor(out=tc_r[:], in0=mean_r[:], in1=sc_r[:],
                                    op=OP.mult)
            nc.vector.tensor_tensor(out=tc_r[:], in0=be_row[:], in1=tc_r[:],
                                    op=OP.subtract)

            # rank-1 broadcast of sc/tc across partitions -> bf16 tiles
            scb_ps = psA.tile([P, P], f32, tag="A")
            nc.tensor.matmul(out=scb_ps[:], lhsT=ones1[:], rhs=sc_r[:],
                             start=True, stop=True)
            sc_bc = constp.tile([P, P], bf16, tag="sc_bc")
            nc.scalar.copy(sc_bc[:], scb_ps[:])
            tcb_ps = psB.tile([P, P], f32, tag="B")
            nc.tensor.matmul(out=tcb_ps[:], lhsT=ones1[:], rhs=tc_r[:],
                             start=True, stop=True)
            tc_bc = constp.tile([P, P], bf16, tag="tc_bc")
            nc.scalar.copy(tc_bc[:], tcb_ps[:])

            # ---- phase B (batched loads/stores, no transposes) -------------
            for bt in range(nt // PBC):
                t0 = bt * PBC
                qa = nc.scalar if bt % 2 == 0 else nc.sync
                qb = nc.sync if bt % 2 == 0 else nc.scalar
                x_b = iop.tile([P, PBC * D], f32, tag="xb")
                qa.dma_start(
                    out=x_b[:].rearrange("p (c e) -> p c e", e=D),
                    in_=xres[:].rearrange("(c p) e -> p c e", p=P)[
                        :, t0:t0 + PBC, :])
                h_b = iop.tile([P, PBC * D], bf16, tag="hb")
                qb.dma_start(
                    out=h_b[:].rearrange("p (c e) -> p c e", e=D),
                    in_=h_d[:].rearrange("(c p) e -> p c e", p=P)[
                        :, t0:t0 + PBC, :])
                g1 = workp.tile([P, PBC * D], bf16, tag="g1")
                nc.vector.tensor_tensor(
                    out=g1[:].rearrange("p (c e) -> p c e", e=D),
                    in0=h_b[:].rearrange("p (c e) -> p c e", e=D),
                    in1=sc_bc[:, None, :].to_broadcast([P, PBC, D]),
                    op=OP.mult)
                nc.vector.tensor_tensor(
                    out=g1[:].rearrange("p (c e) -> p c e", e=D),
                    in0=g1[:].rearrange("p (c e) -> p c e", e=D),
                    in1=tc_bc[:, None, :].to_broadcast([P, PBC, D]),
                    op=OP.add)
                nc.vector.tensor_scalar_max(g1[:], g1[:], 0.0)
                nc.vector.tensor_tensor(out=x_b[:], in0=g1[:], in1=x_b[:],
                                        op=OP.add)
                qb.dma_start(
                    out=out[:].rearrange("(c p) e -> p c e", p=P)[
                        :, t0:t0 + PBC, :],
                    in_=x_b[:].rearrange("p (c e) -> p c e", e=D))

    nc.compile()
    return nc


def _wrap16(a):
    b = a.reshape(-1, 16).T
    return np.tile(b, (8, 1))


def host_prep(x, src, dst, W, b, gamma, beta):
    """Graph routing / layout prep (indices only - no FLOPs on host)."""
    import ml_dtypes

    x = np.asarray(x, np.float32)
    W = np.asarray(W, np.float32)
    gamma = np.asarray(gamma, np.float32)
    beta = np.asarray(beta, np.float32)
    src32 = np.asarray(src).astype(np.int64)
    dst32 = np.asarray(dst).astype(np.int64)

    n = x.shape[0]
    npad = -(-n // (P * NCORES * GROUP)) * (P * NCORES * GROUP)
    nodes_pc = npad // NCORES
    nt = nodes_pc // P
    nt_tot = npad // P
    assert sum(WTILES) == nt
    nwin = len(WTILES)
    wt_start = np.cumsum([0] + list(WTILES))  # in tiles, per core

    order = np.argsort(dst32, kind="stable")
    ds = dst32[order]
    ss = src32[order]

    ar = np.arange(npad + 1, dtype=np.int64)
    rps = np.searchsorted(np.sort(src32), ar).astype(np.int32)
    rpd_full = np.searchsorted(ds, ar)

    # src -> (window, row within window table). Window w of the z table is
    # [core0 quarter w | core1 quarter w | ...], quarter w = tiles
    # [wt_start[w], wt_start[w+1]) of each core's slice.
    s_core = ss // nodes_pc
    s_r = ss % nodes_pc
    s_tile = s_r // P
    s_win = np.searchsorted(wt_start, s_tile, side="right") - 1
    wrows = (np.array(WTILES) * P)[s_win]
    s_winrow = s_core * wrows + (s_r - wt_start[s_win] * P)

    # degree counts (int), F-order [P, nt] per core
    dgo_n = np.diff(rps).astype(np.int32)                 # [npad]
    dgi_n = np.diff(rpd_full).astype(np.int32)            # [npad]

    # per-dst in-degree split by src window
    deg4 = np.zeros((npad, nwin), np.int32)
    np.add.at(deg4, (ds, s_win), 1)

    # --- bin-pack dst nodes into tiles (per core) to flatten the
    # per-(tile, window) edge-count tails, so kws (chunk counts) shrink.
    caps = np.array([3 * P, 5 * P, 5 * P, 5 * P], np.float64)
    newpos = np.empty(npad, np.int64)    # global node -> permuted local slot
    for c in range(NCORES):
        d4 = deg4[c * nodes_pc:(c + 1) * nodes_pc].astype(np.float64)
        order_c = np.argsort(-d4.sum(1), kind="stable")
        loads = np.zeros((nt, nwin))
        counts = np.zeros(nt, np.int64)
        fill = [[] for _ in range(nt)]
        for i in order_c:
            util = np.max((loads + d4[i]) / caps, axis=1)
            util[counts >= P] = np.inf
            b = int(np.argmin(util))
            loads[b] += d4[i]
            counts[b] += 1
            fill[b].append(i)
        pos = np.empty(nodes_pc, np.int64)
        for b in range(nt):
            pos[np.array(fill[b], np.int64)] = (
                b * P + np.arange(len(fill[b])))
        newpos[c * nodes_pc:(c + 1) * nodes_pc] = pos

    # kws from the packed loads (global max over cores/tiles per window)
    e_core = ds // nodes_pc
    e_pos = newpos[ds]
    e_tile = e_core * nt + e_pos // P
    e_slot = e_pos % P
    cell = e_tile * nwin + s_win
    cnt = np.bincount(cell, minlength=nt_tot * nwin).reshape(nt_tot, nwin)
    kws = tuple(int(-(-cnt[:, w].max() // P)) for w in range(nwin))
    ktot = sum(kws)

    # per (tile, window) edge lists under the permutation
    eorder = np.argsort(cell, kind="stable")
    bnd = np.searchsorted(cell[eorder], np.arange(nt_tot * nwin + 1))
    tw_lists = [[eorder[bnd[t * nwin + w]:bnd[t * nwin + w + 1]]
                 for w in range(nwin)] for t in range(nt_tot)]

    xpad = np.zeros((npad, D), np.float32)
    xpad[:n] = x

    ngroups = nt // GROUP
    in_maps = []
    shared = dict(
        wmat=W.astype(ml_dtypes.bfloat16),
        grow=np.ascontiguousarray(gamma[None, :]),
        brow=np.ascontiguousarray(beta[None, :]))
    for c in range(NCORES):
        # eloc layout: (g, w, u, chunk) contiguous for batched expansions
        elocv = np.full((nt * ktot, P), -1.0, np.float32)
        idx_blocks = []
        ecol_off = 0
        for g in range(ngroups):
            for w in range(nwin):
                blk = np.zeros(GROUP * kws[w] * P, np.int16)
                for u in range(GROUP):
                    t = g * GROUP + u
                    gt = c * nt + t
                    sel = tw_lists[gt][w]
                    base = u * kws[w] * P
                    blk[base:base + len(sel)] = s_winrow[sel].astype(np.int16)
                    ev = e_slot[sel].astype(np.float32)
                    ecol = elocv[ecol_off + u * kws[w]:
                                 ecol_off + (u + 1) * kws[w]].reshape(-1)
                    ecol[:len(sel)] = ev
                ecol_off += GROUP * kws[w]
                idx_blocks.append(_wrap16(blk))
        m = dict(shared)
        xslice = xpad[c * nodes_pc:(c + 1) * nodes_pc]
        pos_c = newpos[c * nodes_pc:(c + 1) * nodes_pc]
        orig_of = np.empty(nodes_pc, np.int64)
        orig_of[pos_c] = np.arange(nodes_pc)
        m["xt"] = np.ascontiguousarray(xslice.T).astype(ml_dtypes.bfloat16)
        m["xres"] = np.ascontiguousarray(xslice[orig_of])
        m["dgo"] = np.ascontiguousarray(
            dgo_n[c * nodes_pc:(c + 1) * nodes_pc].reshape(nt, P).T)
        m["dgi"] = np.ascontiguousarray(
            dgi_n[c * nodes_pc:(c + 1) * nodes_pc][orig_of]
            .reshape(nt, P).T)
        m["idxs"] = np.ascontiguousarray(np.concatenate(idx_blocks, axis=1))
        # eloc device layout: col (g,w,u,chunk) partition p = edge c*128+p
        m["eloc"] = np.ascontiguousarray(
            elocv.T).astype(ml_dtypes.bfloat16)
        in_maps.append(m)
    return dict(npad=npad, nt=nt, kws=kws, n_real=n, newpos=newpos), in_maps


def run(in_maps, cfg, **kw):
    from concourse.bass_utils import run_bass_kernel_spmd

    key = (cfg["npad"], cfg["nt"], tuple(cfg["kws"]), cfg["n_real"])
    if key not in _NC_CACHE:
        _NC_CACHE[key] = build_program(*key)
    nc = _NC_CACHE[key]
    res = run_bass_kernel_spmd(nc, in_maps, core_ids=list(range(NCORES)), **kw)
    n = cfg["n_real"]
    nodes_pc = cfg["npad"] // NCORES
    parts = []
    for c in range(NCORES):
        o = np.asarray(res.results[c]["out"])
        pos_c = cfg["newpos"][c * nodes_pc:(c + 1) * nodes_pc]
        parts.append(o[pos_c])
    full = np.concatenate(parts, axis=0)[:n]
    return np.ascontiguousarray(full, dtype=np.float32), res


def kernel(x, src, dst, W, b, gamma, beta):
    cfg, in_maps = host_prep(x, src, dst, W, b, gamma, beta)
    out, _ = run(in_maps, cfg)
    return out


# revision 42
# speedup vs baseline: 1.0489x; 1.0063x over previous
"""DeepGCNLayer (GraphConv norm='both' + BatchNorm + ReLU + residual) on 8 trn2 cores.

Sharding: nodes padded to NPAD=100352, split into 8 ranges (98 node-tiles of
128 per core). Edges routed to the core owning their dst (dst-sorted), then
per (dst-tile, src-window) padded to a uniform K_w chunks of 128 so every
core runs one SPMD program.

v2 layout: the AllGather'd z table is built in 4 row-interleaved windows
(each window = the same quarter of every core's slice) so per-window
AllGathers overlap the z pass and the first gather groups. The one-hot S
matrices are built as ACT-engine broadcast expansion of eloc plus a DVE
is_equal on real tiles (2x perf mode, short shared-port holds) - the v1
broadcast tensor_tensor held the DVE shared SBUF port for ~28us/group,
starving the SWDGE gather descriptor generator (see trainium-docs
memories/01-sbuf.md "DVE blocks DMA" trap). x ships transposed bf16 so the
z pass needs no PE transposes; norm_src/norm_dst fold into ACT scale-copies.
"""

import sys

if "/opt/trn_rl_repo" not in sys.path:
    sys.path.insert(0, "/opt/trn_rl_repo")

import numpy as np

P = 128
D = 128
NCORES = 8
BN_EPS = 1e-5
GROUP = 7            # dst tiles per gather group
PBC = 7              # node tiles per phase-B load/store batch
WTILES = (22, 27, 27, 22)   # z-table window sizes in node tiles (per core)

_NC_CACHE = {}


def build_program(npad, nt, kws, n_real):
    """kws: tuple of chunks-per-window per dst tile (uniform across tiles)."""
    import concourse.bacc as bacc
    import concourse.tile as tile
    from concourse import mybir

    f32 = mybir.dt.float32
    bf16 = mybir.dt.bfloat16
    i32 = mybir.dt.int32
    i16 = mybir.dt.int16
    OP = mybir.AluOpType
    AF = mybir.ActivationFunctionType

    nodes_pc = nt * P
    ktot = sum(kws)
    nwin = len(kws)
    ngroups = nt // GROUP
    assert nt % GROUP == 0
    assert sum(WTILES) == nt and len(WTILES) == nwin
    kmax = max(kws)
    # idx16 columns per (group, window); eloc columns per (group, window)
    cols_gw = [GROUP * kw * P // 16 for kw in kws]
    gcols_i = sum(cols_gw)              # idx cols per group
    gcols_e = GROUP * ktot              # eloc cols per group

    nc = bacc.Bacc("TRN2", target_bir_lowering=False, debug=False,
                   num_devices=NCORES, num_swdge_queues=4)

    xt = nc.dram_tensor("xt", [P, nodes_pc], bf16, kind="ExternalInput")
    xres = nc.dram_tensor("xres", [nodes_pc, D], f32, kind="ExternalInput")
    wmat = nc.dram_tensor("wmat", [D, D], bf16, kind="ExternalInput")
    grow = nc.dram_tensor("grow", [1, D], f32, kind="ExternalInput")
    brow = nc.dram_tensor("brow", [1, D], f32, kind="ExternalInput")
    dgo = nc.dram_tensor("dgo", [P, nt], i32, kind="ExternalInput")
    dgi = nc.dram_tensor("dgi", [P, nt], i32, kind="ExternalInput")
    idxs = nc.dram_tensor("idxs", [P, ngroups * gcols_i], i16,
                          kind="ExternalInput")
    eloc = nc.dram_tensor("eloc", [P, ngroups * gcols_e], bf16,
                          kind="ExternalInput")
    out = nc.dram_tensor("out", [nodes_pc, D], f32, kind="ExternalOutput")
    h_d = nc.dram_tensor("h_d", [nodes_pc, D], bf16)

    # per-window z contribution + AllGather'd table (row-interleaved:
    # window w = [core0 quarter w | core1 quarter w | ...])
    cc_z = [nc.dram_tensor(f"cc_z{w}", [WTILES[w] * P, D], bf16)
            for w in range(nwin)]
    z_t = [nc.dram_tensor(f"z_t{w}", [WTILES[w] * P * NCORES, D], bf16,
                          addr_space="Shared")
           for w in range(nwin)]

    with tile.TileContext(nc) as tc:
        with (
            tc.tile_pool(name="const", bufs=1) as constp,
            tc.tile_pool(name="norm", bufs=1) as normp,
            tc.tile_pool(name="xz", bufs=1) as xzp,
            tc.tile_pool(name="zst", bufs=1) as zsp,
            tc.tile_pool(name="meta", bufs=4) as metap,
            tc.tile_pool(name="gathA", bufs=3) as gathA,
            tc.tile_pool(name="gathB", bufs=3) as gathB,
            tc.tile_pool(name="s", bufs=1) as sp,
            tc.tile_pool(name="work", bufs=2) as workp,
            tc.tile_pool(name="stats", bufs=1) as statp,
            tc.tile_pool(name="io", bufs=3) as iop,
            tc.tile_pool(name="psA", bufs=2, space="PSUM") as psA,
            tc.tile_pool(name="psB", bufs=2, space="PSUM") as psB,
            tc.tile_pool(name="psS", bufs=1, space="PSUM") as psS,
            tc.tile_pool(name="dram", bufs=2, space="DRAM") as dramp,
        ):
            # ---- constants -------------------------------------------------
            iota = constp.tile([P, P], bf16, tag="iota")
            nc.gpsimd.iota(iota[:], pattern=[[1, P]], base=0,
                           channel_multiplier=0,
                           allow_small_or_imprecise_dtypes=True)
            # iota replicated along free dim for batched 2x is_equal
            iota_rep = constp.tile([P, GROUP * kmax * P], bf16, tag="iotar")
            nc.vector.tensor_copy(
                iota_rep[:].rearrange("p (c e) -> p c e", e=P),
                iota[:, None, :].to_broadcast([P, GROUP * kmax, P]))
            ones1 = constp.tile([1, P], f32, tag="ones1")
            nc.vector.memset(ones1[:], 1.0)
            ones_c = constp.tile([P, 1], bf16, tag="ones_c")
            nc.vector.memset(ones_c[:], 1.0)
            w_sb = constp.tile([P, D], bf16, tag="wsb")
            nc.sync.dma_start(out=w_sb[:], in_=wmat[:])
            g_row = constp.tile([1, D], f32, tag="grow")
            nc.sync.dma_start(out=g_row[:], in_=grow[:])
            be_row = constp.tile([1, D], f32, tag="berow")
            nc.sync.dma_start(out=be_row[:], in_=brow[:])

            # ---- norm arrays (own range, F-order [P, nt]) ------------------
            deg = normp.tile([P, nt], i32, tag="deg")
            nc.sync.dma_start(out=deg[:], in_=dgo[:])
            degf = normp.tile([P, nt], f32, tag="degf")
            nc.vector.tensor_scalar_max(degf[:], deg[:], 1.0)
            nc.scalar.sqrt(degf[:], degf[:])
            ns_f = constp.tile([P, nt], f32, tag="ns_f")
            nc.vector.reciprocal(ns_f[:], degf[:])

            deg2 = normp.tile([P, nt], i32, tag="deg2")
            nc.sync.dma_start(out=deg2[:], in_=dgi[:])
            deg2f = normp.tile([P, nt], f32, tag="deg2f")
            nc.vector.tensor_scalar_max(deg2f[:], deg2[:], 1.0)
            nc.scalar.sqrt(deg2f[:], deg2f[:])
            nd_f = constp.tile([P, nt], f32, tag="nd_f")
            nc.vector.reciprocal(nd_f[:], deg2f[:])

            # ---- z pass: z = (x@W)*ns -> bf16, own slice, 4 windows --------
            # lhsT = xT slice (no transposes); ns folds into the ACT
            # PSUM->SBUF copy; each window's AllGather issues as soon as its
            # quarter is stored so window-0 gathers start early.
            t0 = 0
            for w in range(nwin):
                wt = WTILES[w]
                xt_w = xzp.tile([P, wt * P], bf16, tag="xt_w")
                nc.sync.dma_start(out=xt_w[:],
                                  in_=xt[:, t0 * P:(t0 + wt) * P])
                z_w = zsp.tile([P, wt * D], bf16, tag="z_w")
                for c in range(wt):
                    z_ps = psB.tile([P, D], f32, tag="B")
                    nc.tensor.matmul(out=z_ps[:],
                                     lhsT=xt_w[:, c * P:(c + 1) * P],
                                     rhs=w_sb[:], start=True, stop=True)
                    nc.scalar.activation(
                        out=z_w[:, c * D:(c + 1) * D], in_=z_ps[:],
                        func=AF.Copy, scale=ns_f[:, t0 + c:t0 + c + 1])
                nc.scalar.dma_start(
                    out=cc_z[w][:].rearrange("(c p) e -> p c e", p=P),
                    in_=z_w[:].rearrange("p (c e) -> p c e", e=D))
                nc.gpsimd.collective_compute(
                    "AllGather", OP.bypass,
                    replica_groups=[list(range(NCORES))],
                    ins=[cc_z[w][:]], outs=[z_t[w][:]])
                t0 += wt

            # ---- phase A ---------------------------------------------------
            sum_ps = psS.tile([1, P], f32, tag="sum")
            sq_ps = psS.tile([1, P], f32, tag="sq")

            for g in range(ngroups):
                idx_g = metap.tile([P, gcols_i], i16, tag="idxg")
                nc.sync.dma_start(
                    out=idx_g[:], in_=idxs[:, g * gcols_i:(g + 1) * gcols_i])
                eloc_g = metap.tile([P, gcols_e], bf16, tag="elocg")
                nc.sync.dma_start(
                    out=eloc_g[:], in_=eloc[:, g * gcols_e:(g + 1) * gcols_e])

                e_ws = []
                s_ws = []
                ico = 0
                eco = 0
                for w in range(nwin):
                    kw = kws[w]
                    nch = GROUP * kw
                    nidx = nch * P
                    pool = gathA if w < 2 else gathB
                    e_t = pool.tile([P, nch * D], bf16, tag=f"E{w}")
                    nc.gpsimd.dma_gather(
                        e_t[:].rearrange("p (c e) -> p c e", e=D),
                        z_t[w][:],
                        idx_g[:, ico:ico + nidx // 16],
                        nidx, nidx, D, single_packet=False,
                        queue_num=(g + w) % 4)
                    ico += nidx // 16
                    e_ws.append(e_t)
                    s_t = sp.tile([P, nch * P], bf16, tag=f"S{w}")
                    nc.vector.tensor_tensor(
                        out=s_t[:].rearrange("p (c e) -> p c e", e=P),
                        in0=eloc_g[:, eco:eco + nch, None].to_broadcast(
                            [P, nch, P]),
                        in1=iota_rep[:, :nch * P].rearrange(
                            "p (c e) -> p c e", e=P),
                        op=OP.is_equal)
                    eco += nch
                    s_ws.append(s_t)

                h_g = workp.tile([P, GROUP * D], bf16, tag="hg")
                for u in range(GROUP):
                    t = g * GROUP + u
                    agg_ps = psA.tile([P, P], f32, tag="A")
                    ci = 0
                    for w in range(nwin):
                        kw = kws[w]
                        for j in range(kw):
                            nc.tensor.matmul(
                                out=agg_ps[:],
                                lhsT=s_ws[w][:, (u * kw + j) * P:
                                             (u * kw + j + 1) * P],
                                rhs=e_ws[w][:, (u * kw + j) * D:
                                            (u * kw + j + 1) * D],
                                start=(ci + j == 0),
                                stop=(ci + j == ktot - 1))
                        ci += kw
                    h_t = h_g[:, u * D:(u + 1) * D]
                    nc.scalar.activation(out=h_t, in_=agg_ps[:],
                                         func=AF.Copy,
                                         scale=nd_f[:, t:t + 1])
                    sq_sb = workp.tile([P, D], bf16, tag="sqsb")
                    nc.scalar.activation(out=sq_sb[:], in_=h_t,
                                         func=AF.Square)
                    nc.tensor.matmul(out=sum_ps[:], lhsT=ones_c[:], rhs=h_t,
                                     start=(t == 0), stop=(t == nt - 1))
                    nc.tensor.matmul(out=sq_ps[:], lhsT=ones_c[:],
                                     rhs=sq_sb[:],
                                     start=(t == 0), stop=(t == nt - 1))
                nc.scalar.dma_start(
                    out=h_d[:].rearrange("(c p) e -> p c e", p=P)[
                        :, g * GROUP:(g + 1) * GROUP, :],
                    in_=h_g[:].rearrange("p (c e) -> p c e", e=D))

            # ---- BN stats all-reduce + scale/shift (row layout) ------------
            srow = statp.tile([1, 2 * P], f32, tag="srow")
            nc.scalar.copy(srow[0:1, 0:P], sum_ps[:])
            nc.scalar.copy(srow[0:1, P:2 * P], sq_ps[:])
            cc_in = dramp.tile([1, 2 * P], f32, tag="ccin")
            cc_out = dramp.tile([1, 2 * P], f32, tag="ccout")
            nc.gpsimd.dma_start(out=cc_in[:], in_=srow[:])
            nc.gpsimd.collective_compute(
                "AllReduce", OP.add,
                replica_groups=[list(range(NCORES))],
                ins=[cc_in.opt()], outs=[cc_out.opt()])
            grow_sb = statp.tile([1, 2 * P], f32, tag="grow_sb")
            nc.gpsimd.dma_start(out=grow_sb[:], in_=cc_out[:])

            inv_n = 1.0 / float(n_real)
            mean_r = statp.tile([1, P], f32, tag="mean")
            nc.vector.tensor_scalar_mul(mean_r[:], grow_sb[0:1, 0:P], inv_n)
            ex2_r = statp.tile([1, P], f32, tag="ex2")
            nc.vector.tensor_scalar_mul(ex2_r[:], grow_sb[0:1, P:2 * P],
                                        inv_n)
            m2_r = statp.tile([1, P], f32, tag="m2")
            nc.scalar.square(m2_r[:], mean_r[:])
            var_r = statp.tile([1, P], f32, tag="var")
            nc.vector.tensor_tensor(out=var_r[:], in0=ex2_r[:], in1=m2_r[:],
                                    op=OP.subtract)
            nc.vector.tensor_scalar_add(var_r[:], var_r[:], BN_EPS)
            sd_r = statp.tile([1, P], f32, tag="sd")
            nc.scalar.sqrt(sd_r[:], var_r[:])
            inv_r = statp.tile([1, P], f32, tag="inv")
            nc.vector.reciprocal(inv_r[:], sd_r[:])
            sc_r = statp.tile([1, P], f32, tag="sc")
            nc.vector.tensor_tensor(out=sc_r[:], in0=g_row[:], in1=inv_r[:],
                                    op=OP.mult)
            # b cancels in BN: shift = beta - mean*scale
            tc_r = statp.tile([1, P], f32, tag="tc")
            nc.vector.tensor_tensor(out=tc_r[:], in0=mean_r[:], in1=sc_r[:],
                                    op=OP.mult)
            nc.vector.tensor_tensor(out=tc_r[:], in0=be_row[:], in1=tc_r[:],
                                    op=OP.subtract)

            # rank-1 broadcast of sc/tc across partitions -> bf16 tiles
            scb_ps = psA.tile([P, P], f32, tag="A")
            nc.tensor.matmul(out=scb_ps[:], lhsT=ones1[:], rhs=sc_r[:],
                             start=True, stop=True)
            sc_bc = constp.tile([P, P], bf16, tag="sc_bc")
            nc.scalar.copy(sc_bc[:], scb_ps[:])
            tcb_ps = psB.tile([P, P], f32, tag="B")
            nc.tensor.matmul(out=tcb_ps[:], lhsT=ones1[:], rhs=tc_r[:],
                             start=True, stop=True)
            tc_bc = constp.tile([P, P], bf16, tag="tc_bc")
            nc.scalar.copy(tc_bc[:], tcb_ps[:])

            # ---- phase B (batched loads/stores, no transposes) -------------
            for bt in range(nt // PBC):
                t0 = bt * PBC
                qa = nc.scalar if bt % 2 == 0 else nc.sync
                qb = nc.sync if bt % 2 == 0 else nc.scalar
                x_b = iop.tile([P, PBC * D], f32, tag="xb")
                qa.dma_start(
                    out=x_b[:].rearrange("p (c e) -> p c e", e=D),
                    in_=xres[:].rearrange("(c p) e -> p c e", p=P)[
                        :, t0:t0 + PBC, :])
                h_b = iop.tile([P, PBC * D], bf16, tag="hb")
                qb.dma_start(
                    out=h_b[:].rearrange("p (c e) -> p c e", e=D),
                    in_=h_d[:].rearrange("(c p) e -> p c e", p=P)[
                        :, t0:t0 + PBC, :])
                g1 = workp.tile([P, PBC * D], bf16, tag="g1")
                nc.vector.tensor_tensor(
                    out=g1[:].rearrange("p (c e) -> p c e", e=D),
                    in0=h_b[:].rearrange("p (c e) -> p c e", e=D),
                    in1=sc_bc[:, None, :].to_broadcast([P, PBC, D]),
                    op=OP.mult)
                nc.vector.tensor_tensor(
                    out=g1[:].rearrange("p (c e) -> p c e", e=D),
                    in0=g1[:].rearrange("p (c e) -> p c e", e=D),
                    in1=tc_bc[:, None, :].to_broadcast([P, PBC, D]),
                    op=OP.add)
                nc.vector.tensor_scalar_max(g1[:], g1[:], 0.0)
                nc.vector.tensor_tensor(out=x_b[:], in0=g1[:], in1=x_b[:],
                                        op=OP.add)
                qb.dma_start(
                    out=out[:].rearrange("(c p) e -> p c e", p=P)[
                        :, t0:t0 + PBC, :],
                    in_=x_b[:].rearrange("p (c e) -> p c e", e=D))

    nc.compile()
    return nc


def _wrap16(a):
    b = a.reshape(-1, 16).T
    return np.tile(b, (8, 1))


def host_prep(x, src, dst, W, b, gamma, beta):
    """Graph routing / layout prep (indices only - no FLOPs on host)."""
    import ml_dtypes

    x = np.asarray(x, np.float32)
    W = np.asarray(W, np.float32)
    gamma = np.asarray(gamma, np.float32)
    beta = np.asarray(beta, np.float32)
    src32 = np.asarray(src).astype(np.int64)
    dst32 = np.asarray(dst).astype(np.int64)

    n = x.shape[0]
    npad = -(-n // (P * NCORES * GROUP)) * (P * NCORES * GROUP)
    nodes_pc = npad // NCORES
    nt = nodes_pc // P
    nt_tot = npad // P
    assert sum(WTILES) == nt
    nwin = len(WTILES)
    wt_start = np.cumsum([0] + list(WTILES))  # in tiles, per core

    order = np.argsort(dst32, kind="stable")
    ds = dst32[order]
    ss = src32[order]

    ar = np.arange(npad + 1, dtype=np.int64)
    rps = np.searchsorted(np.sort(src32), ar).astype(np.int32)
    rpd_full = np.searchsorted(ds, ar)

    # src -> (window, row within window table). Window w of the z table is
    # [core0 quarter w | core1 quarter w | ...], quarter w = tiles
    # [wt_start[w], wt_start[w+1]) of each core's slice.
    s_core = ss // nodes_pc
    s_r = ss % nodes_pc
    s_tile = s_r // P
    s_win = np.searchsorted(wt_start, s_tile, side="right") - 1
    wrows = (np.array(WTILES) * P)[s_win]
    s_winrow = s_core * wrows + (s_r - wt_start[s_win] * P)

    # degree counts (int), F-order [P, nt] per core
    dgo_n = np.diff(rps).astype(np.int32)                 # [npad]
    dgi_n = np.diff(rpd_full).astype(np.int32)            # [npad]

    # per-dst in-degree split by src window
    deg4 = np.zeros((npad, nwin), np.int32)
    np.add.at(deg4, (ds, s_win), 1)

    # --- bin-pack dst nodes into tiles (per core) to flatten the
    # per-(tile, window) edge-count tails, so kws (chunk counts) shrink.
    caps = np.array([4 * P, 5 * P, 5 * P, 4 * P], np.float64)
    newpos = np.empty(npad, np.int64)    # global node -> permuted local slot
    for c in range(NCORES):
        d4 = deg4[c * nodes_pc:(c + 1) * nodes_pc].astype(np.float64)
        order_c = np.argsort(-d4.sum(1), kind="stable")
        loads = np.zeros((nt, nwin))
        counts = np.zeros(nt, np.int64)
        fill = [[] for _ in range(nt)]
        for i in order_c:
            util = np.max((loads + d4[i]) / caps, axis=1)
            util[counts >= P] = np.inf
            b = int(np.argmin(util))
            loads[b] += d4[i]
            counts[b] += 1
            fill[b].append(i)
        pos = np.empty(nodes_pc, np.int64)
        for b in range(nt):
            pos[np.array(fill[b], np.int64)] = (
                b * P + np.arange(len(fill[b])))
        newpos[c * nodes_pc:(c + 1) * nodes_pc] = pos

    # kws from the packed loads (global max over cores/tiles per window)
    e_core = ds // nodes_pc
    e_pos = newpos[ds]
    e_tile = e_core * nt + e_pos // P
    e_slot = e_pos % P
    cell = e_tile * nwin + s_win
    cnt = np.bincount(cell, minlength=nt_tot * nwin).reshape(nt_tot, nwin)
    kws = tuple(int(-(-cnt[:, w].max() // P)) for w in range(nwin))
    ktot = sum(kws)

    # per (tile, window) edge lists under the permutation
    eorder = np.argsort(cell, kind="stable")
    bnd = np.searchsorted(cell[eorder], np.arange(nt_tot * nwin + 1))
    tw_lists = [[eorder[bnd[t * nwin + w]:bnd[t * nwin + w + 1]]
                 for w in range(nwin)] for t in range(nt_tot)]

    xpad = np.zeros((npad, D), np.float32)
    xpad[:n] = x

    ngroups = nt // GROUP
    in_maps = []
    shared = dict(
        wmat=W.astype(ml_dtypes.bfloat16),
        grow=np.ascontiguousarray(gamma[None, :]),
        brow=np.ascontiguousarray(beta[None, :]))
    for c in range(NCORES):
        # eloc layout: (g, w, u, chunk) contiguous for batched expansions
        elocv = np.full((nt * ktot, P), -1.0, np.float32)
        idx_blocks = []
        ecol_off = 0
        for g in range(ngroups):
            for w in range(nwin):
                blk = np.zeros(GROUP * kws[w] * P, np.int16)
                for u in range(GROUP):
                    t = g * GROUP + u
                    gt = c * nt + t
                    sel = tw_lists[gt][w]
                    base = u * kws[w] * P
                    blk[base:base + len(sel)] = s_winrow[sel].astype(np.int16)
                    ev = e_slot[sel].astype(np.float32)
                    ecol = elocv[ecol_off + u * kws[w]:
                                 ecol_off + (u + 1) * kws[w]].reshape(-1)
                    ecol[:len(sel)] = ev
                ecol_off += GROUP * kws[w]
                idx_blocks.append(_wrap16(blk))
        m = dict(shared)
        xslice = xpad[c * nodes_pc:(c + 1) * nodes_pc]
        pos_c = newpos[c * nodes_pc:(c + 1) * nodes_pc]
        orig_of = np.empty(nodes_pc, np.int64)
        orig_of[pos_c] = np.arange(nodes_pc)
        m["xt"] = np.ascontiguousarray(xslice.T).astype(ml_dtypes.bfloat16)
        m["xres"] = np.ascontiguousarray(xslice[orig_of])
        m["dgo"] = np.ascontiguousarray(
            dgo_n[c * nodes_pc:(c + 1) * nodes_pc].reshape(nt, P).T)
        m["dgi"] = np.ascontiguousarray(
            dgi_n[c * nodes_pc:(c + 1) * nodes_pc][orig_of]
            .reshape(nt, P).T)
        m["idxs"] = np.ascontiguousarray(np.concatenate(idx_blocks, axis=1))
        # eloc device layout: col (g,w,u,chunk) partition p = edge c*128+p
        m["eloc"] = np.ascontiguousarray(
            elocv.T).astype(ml_dtypes.bfloat16)
        in_maps.append(m)
    return dict(npad=npad, nt=nt, kws=kws, n_real=n, newpos=newpos), in_maps


def run(in_maps, cfg, **kw):
    from concourse.bass_utils import run_bass_kernel_spmd

    key = (cfg["npad"], cfg["nt"], tuple(cfg["kws"]), cfg["n_real"])
    if key not in _NC_CACHE:
        _NC_CACHE[key] = build_program(*key)
    nc = _NC_CACHE[key]
    res = run_bass_kernel_spmd(nc, in_maps, core_ids=list(range(NCORES)), **kw)
    n = cfg["n_real"]
    nodes_pc = cfg["npad"] // NCORES
    parts = []
    for c in range(NCORES):
        o = np.asarray(res.results[c]["out"])
        pos_c = cfg["newpos"][c * nodes_pc:(c + 1) * nodes_pc]
        parts.append(o[pos_c])
    full = np.concatenate(parts, axis=0)[:n]
    return np.ascontiguousarray(full, dtype=np.float32), res


def kernel(x, src, dst, W, b, gamma, beta):
    cfg, in_maps = host_prep(x, src, dst, W, b, gamma, beta)
    out, _ = run(in_maps, cfg)
    return out


# revision 44
# speedup vs baseline: 1.1147x; 1.0628x over previous
"""DeepGCNLayer (GraphConv norm='both' + BatchNorm + ReLU + residual) on 8 trn2 cores.

Sharding: nodes padded to NPAD=100352, split into 8 ranges (98 node-tiles of
128 per core). Edges routed to the core owning their dst (dst-sorted), then
per (dst-tile, src-window) padded to a uniform K_w chunks of 128 so every
core runs one SPMD program.

v2 layout: the AllGather'd z table is built in 4 row-interleaved windows
(each window = the same quarter of every core's slice) so per-window
AllGathers overlap the z pass and the first gather groups. The one-hot S
matrices are built as ACT-engine broadcast expansion of eloc plus a DVE
is_equal on real tiles (2x perf mode, short shared-port holds) - the v1
broadcast tensor_tensor held the DVE shared SBUF port for ~28us/group,
starving the SWDGE gather descriptor generator (see trainium-docs
memories/01-sbuf.md "DVE blocks DMA" trap). x ships transposed bf16 so the
z pass needs no PE transposes; norm_src/norm_dst fold into ACT scale-copies.
"""

import sys

if "/opt/trn_rl_repo" not in sys.path:
    sys.path.insert(0, "/opt/trn_rl_repo")

import numpy as np

P = 128
D = 128
NCORES = 8
BN_EPS = 1e-5
GROUP = 7            # dst tiles per gather group
PBC = 7              # node tiles per phase-B load/store batch
WTILES = (22, 27, 27, 22)   # z-table window sizes in node tiles (per core)

_NC_CACHE = {}


def build_program(npad, nt, kws, n_real):
    """kws: tuple of chunks-per-window per dst tile (uniform across tiles)."""
    import concourse.bacc as bacc
    import concourse.tile as tile
    from concourse import mybir

    f32 = mybir.dt.float32
    bf16 = mybir.dt.bfloat16
    i32 = mybir.dt.int32
    i16 = mybir.dt.int16
    OP = mybir.AluOpType
    AF = mybir.ActivationFunctionType

    nodes_pc = nt * P
    ktot = sum(kws)
    nwin = len(kws)
    ngroups = nt // GROUP
    assert nt % GROUP == 0
    assert sum(WTILES) == nt and len(WTILES) == nwin
    kmax = max(kws)
    # idx16 columns per (group, window); eloc columns per (group, window)
    cols_gw = [GROUP * kw * P // 16 for kw in kws]
    gcols_i = sum(cols_gw)              # idx cols per group
    gcols_e = GROUP * ktot              # eloc cols per group

    nc = bacc.Bacc("TRN2", target_bir_lowering=False, debug=False,
                   num_devices=NCORES, num_swdge_queues=4)

    xt = nc.dram_tensor("xt", [P, nodes_pc], bf16, kind="ExternalInput")
    xres = nc.dram_tensor("xres", [nodes_pc, D], f32, kind="ExternalInput")
    wmat = nc.dram_tensor("wmat", [D, D], bf16, kind="ExternalInput")
    grow = nc.dram_tensor("grow", [1, D], f32, kind="ExternalInput")
    brow = nc.dram_tensor("brow", [1, D], f32, kind="ExternalInput")
    dgo = nc.dram_tensor("dgo", [P, nt], i32, kind="ExternalInput")
    dgi = nc.dram_tensor("dgi", [P, nt], i32, kind="ExternalInput")
    idxs = nc.dram_tensor("idxs", [P, ngroups * gcols_i], i16,
                          kind="ExternalInput")
    eloc = nc.dram_tensor("eloc", [P, ngroups * gcols_e], bf16,
                          kind="ExternalInput")
    out = nc.dram_tensor("out", [nodes_pc, D], f32, kind="ExternalOutput")
    h_d = nc.dram_tensor("h_d", [nodes_pc, D], bf16)

    # per-window z contribution + AllGather'd table (row-interleaved:
    # window w = [core0 quarter w | core1 quarter w | ...])
    cc_z = [nc.dram_tensor(f"cc_z{w}", [WTILES[w] * P, D], bf16)
            for w in range(nwin)]
    z_t = [nc.dram_tensor(f"z_t{w}", [WTILES[w] * P * NCORES, D], bf16,
                          addr_space="Shared")
           for w in range(nwin)]

    with tile.TileContext(nc) as tc:
        with (
            tc.tile_pool(name="const", bufs=1) as constp,
            tc.tile_pool(name="norm", bufs=1) as normp,
            tc.tile_pool(name="xz", bufs=1) as xzp,
            tc.tile_pool(name="zst", bufs=1) as zsp,
            tc.tile_pool(name="meta", bufs=6) as metap,
            tc.tile_pool(name="gathA", bufs=3) as gathA,
            tc.tile_pool(name="gathB", bufs=3) as gathB,
            tc.tile_pool(name="s", bufs=1) as sp,
            tc.tile_pool(name="work", bufs=2) as workp,
            tc.tile_pool(name="stats", bufs=1) as statp,
            tc.tile_pool(name="io", bufs=3) as iop,
            tc.tile_pool(name="psA", bufs=2, space="PSUM") as psA,
            tc.tile_pool(name="psB", bufs=2, space="PSUM") as psB,
            tc.tile_pool(name="psS", bufs=1, space="PSUM") as psS,
            tc.tile_pool(name="dram", bufs=2, space="DRAM") as dramp,
        ):
            # ---- constants -------------------------------------------------
            iota = constp.tile([P, P], bf16, tag="iota")
            nc.gpsimd.iota(iota[:], pattern=[[1, P]], base=0,
                           channel_multiplier=0,
                           allow_small_or_imprecise_dtypes=True)
            # iota replicated along free dim for batched 2x is_equal
            iota_rep = constp.tile([P, GROUP * kmax * P], bf16, tag="iotar")
            nc.vector.tensor_copy(
                iota_rep[:].rearrange("p (c e) -> p c e", e=P),
                iota[:, None, :].to_broadcast([P, GROUP * kmax, P]))
            ones1 = constp.tile([1, P], f32, tag="ones1")
            nc.vector.memset(ones1[:], 1.0)
            ones_c = constp.tile([P, 1], bf16, tag="ones_c")
            nc.vector.memset(ones_c[:], 1.0)
            w_sb = constp.tile([P, D], bf16, tag="wsb")
            nc.sync.dma_start(out=w_sb[:], in_=wmat[:])
            g_row = constp.tile([1, D], f32, tag="grow")
            nc.sync.dma_start(out=g_row[:], in_=grow[:])
            be_row = constp.tile([1, D], f32, tag="berow")
            nc.sync.dma_start(out=be_row[:], in_=brow[:])

            # ---- norm arrays (own range, F-order [P, nt]) ------------------
            deg = normp.tile([P, nt], i32, tag="deg")
            nc.sync.dma_start(out=deg[:], in_=dgo[:])
            degf = normp.tile([P, nt], f32, tag="degf")
            nc.vector.tensor_scalar_max(degf[:], deg[:], 1.0)
            nc.scalar.sqrt(degf[:], degf[:])
            ns_f = constp.tile([P, nt], f32, tag="ns_f")
            nc.vector.reciprocal(ns_f[:], degf[:])

            deg2 = normp.tile([P, nt], i32, tag="deg2")
            nc.sync.dma_start(out=deg2[:], in_=dgi[:])
            deg2f = normp.tile([P, nt], f32, tag="deg2f")
            nc.vector.tensor_scalar_max(deg2f[:], deg2[:], 1.0)
            nc.scalar.sqrt(deg2f[:], deg2f[:])
            nd_f = constp.tile([P, nt], f32, tag="nd_f")
            nc.vector.reciprocal(nd_f[:], deg2f[:])

            # ---- z pass: z = (x@W)*ns -> bf16, own slice, 4 windows --------
            # lhsT = xT slice (no transposes); ns folds into the ACT
            # PSUM->SBUF copy; each window's AllGather issues as soon as its
            # quarter is stored so window-0 gathers start early.
            t0 = 0
            for w in range(nwin):
                wt = WTILES[w]
                xt_w = xzp.tile([P, wt * P], bf16, tag="xt_w")
                nc.sync.dma_start(out=xt_w[:],
                                  in_=xt[:, t0 * P:(t0 + wt) * P])
                z_w = zsp.tile([P, wt * D], bf16, tag="z_w")
                for c in range(wt):
                    z_ps = psB.tile([P, D], f32, tag="B")
                    nc.tensor.matmul(out=z_ps[:],
                                     lhsT=xt_w[:, c * P:(c + 1) * P],
                                     rhs=w_sb[:], start=True, stop=True)
                    nc.scalar.activation(
                        out=z_w[:, c * D:(c + 1) * D], in_=z_ps[:],
                        func=AF.Copy, scale=ns_f[:, t0 + c:t0 + c + 1])
                nc.scalar.dma_start(
                    out=cc_z[w][:].rearrange("(c p) e -> p c e", p=P),
                    in_=z_w[:].rearrange("p (c e) -> p c e", e=D))
                nc.gpsimd.collective_compute(
                    "AllGather", OP.bypass,
                    replica_groups=[list(range(NCORES))],
                    ins=[cc_z[w][:]], outs=[z_t[w][:]])
                t0 += wt

            # ---- phase A ---------------------------------------------------
            sum_ps = psS.tile([1, P], f32, tag="sum")
            sq_ps = psS.tile([1, P], f32, tag="sq")

            for g in range(ngroups):
                idx_g = metap.tile([P, gcols_i], i16, tag="idxg")
                nc.sync.dma_start(
                    out=idx_g[:], in_=idxs[:, g * gcols_i:(g + 1) * gcols_i])
                eloc_g = metap.tile([P, gcols_e], bf16, tag="elocg")
                nc.sync.dma_start(
                    out=eloc_g[:], in_=eloc[:, g * gcols_e:(g + 1) * gcols_e])

                e_ws = []
                s_ws = []
                ico = 0
                eco = 0
                for w in range(nwin):
                    kw = kws[w]
                    nch = GROUP * kw
                    nidx = nch * P
                    pool = gathA if w < 2 else gathB
                    e_t = pool.tile([P, nch * D], bf16, tag=f"E{w}")
                    nc.gpsimd.dma_gather(
                        e_t[:].rearrange("p (c e) -> p c e", e=D),
                        z_t[w][:],
                        idx_g[:, ico:ico + nidx // 16],
                        nidx, nidx, D, single_packet=False,
                        queue_num=(g + w) % 4)
                    ico += nidx // 16
                    e_ws.append(e_t)
                    s_t = sp.tile([P, nch * P], bf16, tag=f"S{w}")
                    nc.vector.tensor_tensor(
                        out=s_t[:].rearrange("p (c e) -> p c e", e=P),
                        in0=eloc_g[:, eco:eco + nch, None].to_broadcast(
                            [P, nch, P]),
                        in1=iota_rep[:, :nch * P].rearrange(
                            "p (c e) -> p c e", e=P),
                        op=OP.is_equal)
                    eco += nch
                    s_ws.append(s_t)

                h_g = workp.tile([P, GROUP * D], bf16, tag="hg")
                for u in range(GROUP):
                    t = g * GROUP + u
                    agg_ps = psA.tile([P, P], f32, tag="A")
                    ci = 0
                    for w in range(nwin):
                        kw = kws[w]
                        for j in range(kw):
                            nc.tensor.matmul(
                                out=agg_ps[:],
                                lhsT=s_ws[w][:, (u * kw + j) * P:
                                             (u * kw + j + 1) * P],
                                rhs=e_ws[w][:, (u * kw + j) * D:
                                            (u * kw + j + 1) * D],
                                start=(ci + j == 0),
                                stop=(ci + j == ktot - 1))
                        ci += kw
                    h_t = h_g[:, u * D:(u + 1) * D]
                    nc.scalar.activation(out=h_t, in_=agg_ps[:],
                                         func=AF.Copy,
                                         scale=nd_f[:, t:t + 1])
                    sq_sb = workp.tile([P, D], bf16, tag="sqsb")
                    nc.scalar.activation(out=sq_sb[:], in_=h_t,
                                         func=AF.Square)
                    nc.tensor.matmul(out=sum_ps[:], lhsT=ones_c[:], rhs=h_t,
                                     start=(t == 0), stop=(t == nt - 1))
                    nc.tensor.matmul(out=sq_ps[:], lhsT=ones_c[:],
                                     rhs=sq_sb[:],
                                     start=(t == 0), stop=(t == nt - 1))
                nc.scalar.dma_start(
                    out=h_d[:].rearrange("(c p) e -> p c e", p=P)[
                        :, g * GROUP:(g + 1) * GROUP, :],
                    in_=h_g[:].rearrange("p (c e) -> p c e", e=D))

            # ---- BN stats all-reduce + scale/shift (row layout) ------------
            srow = statp.tile([1, 2 * P], f32, tag="srow")
            nc.scalar.copy(srow[0:1, 0:P], sum_ps[:])
            nc.scalar.copy(srow[0:1, P:2 * P], sq_ps[:])
            cc_in = dramp.tile([1, 2 * P], f32, tag="ccin")
            cc_out = dramp.tile([1, 2 * P], f32, tag="ccout")
            nc.gpsimd.dma_start(out=cc_in[:], in_=srow[:])
            nc.gpsimd.collective_compute(
                "AllReduce", OP.add,
                replica_groups=[list(range(NCORES))],
                ins=[cc_in.opt()], outs=[cc_out.opt()])
            grow_sb = statp.tile([1, 2 * P], f32, tag="grow_sb")
            nc.gpsimd.dma_start(out=grow_sb[:], in_=cc_out[:])

            inv_n = 1.0 / float(n_real)
            mean_r = statp.tile([1, P], f32, tag="mean")
            nc.vector.tensor_scalar_mul(mean_r[:], grow_sb[0:1, 0:P], inv_n)
            ex2_r = statp.tile([1, P], f32, tag="ex2")
            nc.vector.tensor_scalar_mul(ex2_r[:], grow_sb[0:1, P:2 * P],
                                        inv_n)
            m2_r = statp.tile([1, P], f32, tag="m2")
            nc.scalar.square(m2_r[:], mean_r[:])
            var_r = statp.tile([1, P], f32, tag="var")
            nc.vector.tensor_tensor(out=var_r[:], in0=ex2_r[:], in1=m2_r[:],
                                    op=OP.subtract)
            nc.vector.tensor_scalar_add(var_r[:], var_r[:], BN_EPS)
            sd_r = statp.tile([1, P], f32, tag="sd")
            nc.scalar.sqrt(sd_r[:], var_r[:])
            inv_r = statp.tile([1, P], f32, tag="inv")
            nc.vector.reciprocal(inv_r[:], sd_r[:])
            sc_r = statp.tile([1, P], f32, tag="sc")
            nc.vector.tensor_tensor(out=sc_r[:], in0=g_row[:], in1=inv_r[:],
                                    op=OP.mult)
            # b cancels in BN: shift = beta - mean*scale
            tc_r = statp.tile([1, P], f32, tag="tc")
            nc.vector.tensor_tensor(out=tc_r[:], in0=mean_r[:], in1=sc_r[:],
                                    op=OP.mult)
            nc.vector.tensor_tensor(out=tc_r[:], in0=be_row[:], in1=tc_r[:],
                                    op=OP.subtract)

            # rank-1 broadcast of sc/tc across partitions -> bf16 tiles
            scb_ps = psA.tile([P, P], f32, tag="A")
            nc.tensor.matmul(out=scb_ps[:], lhsT=ones1[:], rhs=sc_r[:],
                             start=True, stop=True)
            sc_bc = constp.tile([P, P], bf16, tag="sc_bc")
            nc.scalar.copy(sc_bc[:], scb_ps[:])
            tcb_ps = psB.tile([P, P], f32, tag="B")
            nc.tensor.matmul(out=tcb_ps[:], lhsT=ones1[:], rhs=tc_r[:],
                             start=True, stop=True)
            tc_bc = constp.tile([P, P], bf16, tag="tc_bc")
            nc.scalar.copy(tc_bc[:], tcb_ps[:])

            # ---- phase B (batched loads/stores, no transposes) -------------
            for bt in range(nt // PBC):
                t0 = bt * PBC
                qa = nc.scalar if bt % 2 == 0 else nc.sync
                qb = nc.sync if bt % 2 == 0 else nc.scalar
                x_b = iop.tile([P, PBC * D], f32, tag="xb")
                qa.dma_start(
                    out=x_b[:].rearrange("p (c e) -> p c e", e=D),
                    in_=xres[:].rearrange("(c p) e -> p c e", p=P)[
                        :, t0:t0 + PBC, :])
                h_b = iop.tile([P, PBC * D], bf16, tag="hb")
                qb.dma_start(
                    out=h_b[:].rearrange("p (c e) -> p c e", e=D),
                    in_=h_d[:].rearrange("(c p) e -> p c e", p=P)[
                        :, t0:t0 + PBC, :])
                g1 = workp.tile([P, PBC * D], bf16, tag="g1")
                nc.vector.tensor_tensor(
                    out=g1[:].rearrange("p (c e) -> p c e", e=D),
                    in0=h_b[:].rearrange("p (c e) -> p c e", e=D),
                    in1=sc_bc[:, None, :].to_broadcast([P, PBC, D]),
                    op=OP.mult)
                nc.vector.tensor_tensor(
                    out=g1[:].rearrange("p (c e) -> p c e", e=D),
                    in0=g1[:].rearrange("p (c e) -> p c e", e=D),
                    in1=tc_bc[:, None, :].to_broadcast([P, PBC, D]),
                    op=OP.add)
                nc.vector.scalar_tensor_tensor(
                    out=x_b[:], in0=g1[:], scalar=0.0, in1=x_b[:],
                    op0=OP.max, op1=OP.add)
                qb.dma_start(
                    out=out[:].rearrange("(c p) e -> p c e", p=P)[
                        :, t0:t0 + PBC, :],
                    in_=x_b[:].rearrange("p (c e) -> p c e", e=D))

    nc.compile()
    return nc


def _wrap16(a):
    b = a.reshape(-1, 16).T
    return np.tile(b, (8, 1))


def host_prep(x, src, dst, W, b, gamma, beta):
    """Graph routing / layout prep (indices only - no FLOPs on host)."""
    import ml_dtypes

    x = np.asarray(x, np.float32)
    W = np.asarray(W, np.float32)
    gamma = np.asarray(gamma, np.float32)
    beta = np.asarray(beta, np.float32)
    src32 = np.asarray(src).astype(np.int64)
    dst32 = np.asarray(dst).astype(np.int64)

    n = x.shape[0]
    npad = -(-n // (P * NCORES * GROUP)) * (P * NCORES * GROUP)
    nodes_pc = npad // NCORES
    nt = nodes_pc // P
    nt_tot = npad // P
    assert sum(WTILES) == nt
    nwin = len(WTILES)
    wt_start = np.cumsum([0] + list(WTILES))  # in tiles, per core

    order = np.argsort(dst32, kind="stable")
    ds = dst32[order]
    ss = src32[order]

    ar = np.arange(npad + 1, dtype=np.int64)
    rps = np.searchsorted(np.sort(src32), ar).astype(np.int32)
    rpd_full = np.searchsorted(ds, ar)

    # src -> (window, row within window table). Window w of the z table is
    # [core0 quarter w | core1 quarter w | ...], quarter w = tiles
    # [wt_start[w], wt_start[w+1]) of each core's slice.
    s_core = ss // nodes_pc
    s_r = ss % nodes_pc
    s_tile = s_r // P
    s_win = np.searchsorted(wt_start, s_tile, side="right") - 1
    wrows = (np.array(WTILES) * P)[s_win]
    s_winrow = s_core * wrows + (s_r - wt_start[s_win] * P)

    # degree counts (int), F-order [P, nt] per core
    dgo_n = np.diff(rps).astype(np.int32)                 # [npad]
    dgi_n = np.diff(rpd_full).astype(np.int32)            # [npad]

    # per-dst in-degree split by src window
    deg4 = np.zeros((npad, nwin), np.int32)
    np.add.at(deg4, (ds, s_win), 1)

    # --- bin-pack dst nodes into tiles (per core) to flatten the
    # per-(tile, window) edge-count tails, so kws (chunk counts) shrink.
    caps = np.array([4 * P, 5 * P, 5 * P, 4 * P], np.float64)
    newpos = np.empty(npad, np.int64)    # global node -> permuted local slot
    for c in range(NCORES):
        d4 = deg4[c * nodes_pc:(c + 1) * nodes_pc].astype(np.float64)
        order_c = np.argsort(-d4.sum(1), kind="stable")
        loads = np.zeros((nt, nwin))
        counts = np.zeros(nt, np.int64)
        fill = [[] for _ in range(nt)]
        for i in order_c:
            util = np.max((loads + d4[i]) / caps, axis=1)
            util[counts >= P] = np.inf
            b = int(np.argmin(util))
            loads[b] += d4[i]
            counts[b] += 1
            fill[b].append(i)
        pos = np.empty(nodes_pc, np.int64)
        for b in range(nt):
            pos[np.array(fill[b], np.int64)] = (
                b * P + np.arange(len(fill[b])))
        newpos[c * nodes_pc:(c + 1) * nodes_pc] = pos

    # kws from the packed loads (global max over cores/tiles per window)
    e_core = ds // nodes_pc
    e_pos = newpos[ds]
    e_tile = e_core * nt + e_pos // P
    e_slot = e_pos % P
    cell = e_tile * nwin + s_win
    cnt = np.bincount(cell, minlength=nt_tot * nwin).reshape(nt_tot, nwin)
    kws = tuple(int(-(-cnt[:, w].max() // P)) for w in range(nwin))
    ktot = sum(kws)

    # per (tile, window) edge lists under the permutation
    eorder = np.argsort(cell, kind="stable")
    bnd = np.searchsorted(cell[eorder], np.arange(nt_tot * nwin + 1))
    tw_lists = [[eorder[bnd[t * nwin + w]:bnd[t * nwin + w + 1]]
                 for w in range(nwin)] for t in range(nt_tot)]

    xpad = np.zeros((npad, D), np.float32)
    xpad[:n] = x

    ngroups = nt // GROUP
    in_maps = []
    shared = dict(
        wmat=W.astype(ml_dtypes.bfloat16),
        grow=np.ascontiguousarray(gamma[None, :]),
        brow=np.ascontiguousarray(beta[None, :]))
    for c in range(NCORES):
        # eloc layout: (g, w, u, chunk) contiguous for batched expansions
        elocv = np.full((nt * ktot, P), -1.0, np.float32)
        idx_blocks = []
        ecol_off = 0
        for g in range(ngroups):
            for w in range(nwin):
                blk = np.zeros(GROUP * kws[w] * P, np.int16)
                for u in range(GROUP):
                    t = g * GROUP + u
                    gt = c * nt + t
                    sel = tw_lists[gt][w]
                    base = u * kws[w] * P
                    blk[base:base + len(sel)] = s_winrow[sel].astype(np.int16)
                    ev = e_slot[sel].astype(np.float32)
                    ecol = elocv[ecol_off + u * kws[w]:
                                 ecol_off + (u + 1) * kws[w]].reshape(-1)
                    ecol[:len(sel)] = ev
                ecol_off += GROUP * kws[w]
                idx_blocks.append(_wrap16(blk))
        m = dict(shared)
        xslice = xpad[c * nodes_pc:(c + 1) * nodes_pc]
        pos_c = newpos[c * nodes_pc:(c + 1) * nodes_pc]
        orig_of = np.empty(nodes_pc, np.int64)
        orig_of[pos_c] = np.arange(nodes_pc)
        m["xt"] = np.ascontiguousarray(xslice.T).astype(ml_dtypes.bfloat16)
        m["xres"] = np.ascontiguousarray(xslice[orig_of])
        m["dgo"] = np.ascontiguousarray(
            dgo_n[c * nodes_pc:(c + 1) * nodes_pc].reshape(nt, P).T)
        m["dgi"] = np.ascontiguousarray(
            dgi_n[c * nodes_pc:(c + 1) * nodes_pc][orig_of]
            .reshape(nt, P).T)
        m["idxs"] = np.ascontiguousarray(np.concatenate(idx_blocks, axis=1))
        # eloc device layout: col (g,w,u,chunk) partition p = edge c*128+p
        m["eloc"] = np.ascontiguousarray(
            elocv.T).astype(ml_dtypes.bfloat16)
        in_maps.append(m)
    return dict(npad=npad, nt=nt, kws=kws, n_real=n, newpos=newpos), in_maps


def run(in_maps, cfg, **kw):
    from concourse.bass_utils import run_bass_kernel_spmd

    key = (cfg["npad"], cfg["nt"], tuple(cfg["kws"]), cfg["n_real"])
    if key not in _NC_CACHE:
        _NC_CACHE[key] = build_program(*key)
    nc = _NC_CACHE[key]
    res = run_bass_kernel_spmd(nc, in_maps, core_ids=list(range(NCORES)), **kw)
    n = cfg["n_real"]
    nodes_pc = cfg["npad"] // NCORES
    parts = []
    for c in range(NCORES):
        o = np.asarray(res.results[c]["out"])
        pos_c = cfg["newpos"][c * nodes_pc:(c + 1) * nodes_pc]
        parts.append(o[pos_c])
    full = np.concatenate(parts, axis=0)[:n]
    return np.ascontiguousarray(full, dtype=np.float32), res


def kernel(x, src, dst, W, b, gamma, beta):
    cfg, in_maps = host_prep(x, src, dst, W, b, gamma, beta)
    out, _ = run(in_maps, cfg)
    return out


# revision 45
# speedup vs baseline: 1.1264x; 1.0105x over previous
"""DeepGCNLayer (GraphConv norm='both' + BatchNorm + ReLU + residual) on 8 trn2 cores.

Sharding: nodes padded to NPAD=100352, split into 8 ranges (98 node-tiles of
128 per core). Edges routed to the core owning their dst (dst-sorted), then
per (dst-tile, src-window) padded to a uniform K_w chunks of 128 so every
core runs one SPMD program.

v2 layout: the AllGather'd z table is built in 4 row-interleaved windows
(each window = the same quarter of every core's slice) so per-window
AllGathers overlap the z pass and the first gather groups. The one-hot S
matrices are built as ACT-engine broadcast expansion of eloc plus a DVE
is_equal on real tiles (2x perf mode, short shared-port holds) - the v1
broadcast tensor_tensor held the DVE shared SBUF port for ~28us/group,
starving the SWDGE gather descriptor generator (see trainium-docs
memories/01-sbuf.md "DVE blocks DMA" trap). x ships transposed bf16 so the
z pass needs no PE transposes; norm_src/norm_dst fold into ACT scale-copies.
"""

import sys

if "/opt/trn_rl_repo" not in sys.path:
    sys.path.insert(0, "/opt/trn_rl_repo")

import numpy as np

P = 128
D = 128
NCORES = 8
BN_EPS = 1e-5
GROUP = 7            # dst tiles per gather group
PBC = 7              # node tiles per phase-B load/store batch
WTILES = (22, 27, 27, 22)   # z-table window sizes in node tiles (per core)

_NC_CACHE = {}


def build_program(npad, nt, kws, n_real):
    """kws: tuple of chunks-per-window per dst tile (uniform across tiles)."""
    import concourse.bacc as bacc
    import concourse.tile as tile
    from concourse import mybir

    f32 = mybir.dt.float32
    bf16 = mybir.dt.bfloat16
    i32 = mybir.dt.int32
    i16 = mybir.dt.int16
    OP = mybir.AluOpType
    AF = mybir.ActivationFunctionType

    nodes_pc = nt * P
    ktot = sum(kws)
    nwin = len(kws)
    ngroups = nt // GROUP
    assert nt % GROUP == 0
    assert sum(WTILES) == nt and len(WTILES) == nwin
    kmax = max(kws)
    # idx16 columns per (group, window); eloc columns per (group, window)
    cols_gw = [GROUP * kw * P // 16 for kw in kws]
    gcols_i = sum(cols_gw)              # idx cols per group
    gcols_e = GROUP * ktot              # eloc cols per group

    nc = bacc.Bacc("TRN2", target_bir_lowering=False, debug=False,
                   num_devices=NCORES, num_swdge_queues=4)

    xt = nc.dram_tensor("xt", [P, nodes_pc], bf16, kind="ExternalInput")
    xres = nc.dram_tensor("xres", [nodes_pc, D], f32, kind="ExternalInput")
    wmat = nc.dram_tensor("wmat", [D, D], bf16, kind="ExternalInput")
    grow = nc.dram_tensor("grow", [1, D], f32, kind="ExternalInput")
    brow = nc.dram_tensor("brow", [1, D], f32, kind="ExternalInput")
    dgo = nc.dram_tensor("dgo", [P, nt], i32, kind="ExternalInput")
    dgi = nc.dram_tensor("dgi", [P, nt], i32, kind="ExternalInput")
    idxs = nc.dram_tensor("idxs", [P, ngroups * gcols_i], i16,
                          kind="ExternalInput")
    eloc = nc.dram_tensor("eloc", [P, ngroups * gcols_e], bf16,
                          kind="ExternalInput")
    out = nc.dram_tensor("out", [nodes_pc, D], f32, kind="ExternalOutput")
    h_d = nc.dram_tensor("h_d", [nodes_pc, D], bf16)

    # per-window z contribution + AllGather'd table (row-interleaved:
    # window w = [core0 quarter w | core1 quarter w | ...])
    cc_z = [nc.dram_tensor(f"cc_z{w}", [WTILES[w] * P, D], bf16)
            for w in range(nwin)]
    z_t = [nc.dram_tensor(f"z_t{w}", [WTILES[w] * P * NCORES, D], bf16,
                          addr_space="Shared")
           for w in range(nwin)]

    with tile.TileContext(nc) as tc:
        with (
            tc.tile_pool(name="const", bufs=1) as constp,
            tc.tile_pool(name="norm", bufs=1) as normp,
            tc.tile_pool(name="xz", bufs=1) as xzp,
            tc.tile_pool(name="zst", bufs=1) as zsp,
            tc.tile_pool(name="meta", bufs=6) as metap,
            tc.tile_pool(name="gathA", bufs=3) as gathA,
            tc.tile_pool(name="gathB", bufs=3) as gathB,
            tc.tile_pool(name="s", bufs=1) as sp,
            tc.tile_pool(name="work", bufs=2) as workp,
            tc.tile_pool(name="stats", bufs=1) as statp,
            tc.tile_pool(name="io", bufs=3) as iop,
            tc.tile_pool(name="psA", bufs=2, space="PSUM") as psA,
            tc.tile_pool(name="psB", bufs=2, space="PSUM") as psB,
            tc.tile_pool(name="psS", bufs=1, space="PSUM") as psS,
            tc.tile_pool(name="dram", bufs=2, space="DRAM") as dramp,
        ):
            # ---- constants -------------------------------------------------
            iota = constp.tile([P, P], bf16, tag="iota")
            nc.gpsimd.iota(iota[:], pattern=[[1, P]], base=0,
                           channel_multiplier=0,
                           allow_small_or_imprecise_dtypes=True)
            # iota replicated along free dim for batched 2x is_equal
            iota_rep = constp.tile([P, GROUP * kmax * P], bf16, tag="iotar")
            nc.vector.tensor_copy(
                iota_rep[:].rearrange("p (c e) -> p c e", e=P),
                iota[:, None, :].to_broadcast([P, GROUP * kmax, P]))
            ones1 = constp.tile([1, P], f32, tag="ones1")
            nc.vector.memset(ones1[:], 1.0)
            ones_c = constp.tile([P, 1], bf16, tag="ones_c")
            nc.vector.memset(ones_c[:], 1.0)
            w_sb = constp.tile([P, D], bf16, tag="wsb")
            nc.sync.dma_start(out=w_sb[:], in_=wmat[:])
            g_row = constp.tile([1, D], f32, tag="grow")
            nc.sync.dma_start(out=g_row[:], in_=grow[:])
            be_row = constp.tile([1, D], f32, tag="berow")
            nc.sync.dma_start(out=be_row[:], in_=brow[:])

            # ---- norm arrays (own range, F-order [P, nt]) ------------------
            deg = normp.tile([P, nt], i32, tag="deg")
            nc.sync.dma_start(out=deg[:], in_=dgo[:])
            degf = normp.tile([P, nt], f32, tag="degf")
            nc.vector.tensor_scalar_max(degf[:], deg[:], 1.0)
            nc.scalar.sqrt(degf[:], degf[:])
            ns_f = constp.tile([P, nt], f32, tag="ns_f")
            nc.vector.reciprocal(ns_f[:], degf[:])

            deg2 = normp.tile([P, nt], i32, tag="deg2")
            nc.sync.dma_start(out=deg2[:], in_=dgi[:])
            deg2f = normp.tile([P, nt], f32, tag="deg2f")
            nc.vector.tensor_scalar_max(deg2f[:], deg2[:], 1.0)
            nc.scalar.sqrt(deg2f[:], deg2f[:])
            nd_f = constp.tile([P, nt], f32, tag="nd_f")
            nc.vector.reciprocal(nd_f[:], deg2f[:])

            # ---- z pass: z = (x@W)*ns -> bf16, own slice, 4 windows --------
            # lhsT = xT slice (no transposes); ns folds into the ACT
            # PSUM->SBUF copy; each window's AllGather issues as soon as its
            # quarter is stored so window-0 gathers start early.
            t0 = 0
            for w in range(nwin):
                wt = WTILES[w]
                xt_w = xzp.tile([P, wt * P], bf16, tag="xt_w")
                nc.sync.dma_start(out=xt_w[:],
                                  in_=xt[:, t0 * P:(t0 + wt) * P])
                z_w = zsp.tile([P, wt * D], bf16, tag="z_w")
                for c in range(wt):
                    z_ps = psB.tile([P, D], f32, tag="B")
                    nc.tensor.matmul(out=z_ps[:],
                                     lhsT=xt_w[:, c * P:(c + 1) * P],
                                     rhs=w_sb[:], start=True, stop=True)
                    if c % 2 == 0:
                        nc.scalar.activation(
                            out=z_w[:, c * D:(c + 1) * D], in_=z_ps[:],
                            func=AF.Copy, scale=ns_f[:, t0 + c:t0 + c + 1])
                    else:
                        nc.vector.tensor_scalar_mul(
                            z_w[:, c * D:(c + 1) * D], z_ps[:],
                            ns_f[:, t0 + c:t0 + c + 1])
                nc.scalar.dma_start(
                    out=cc_z[w][:].rearrange("(c p) e -> p c e", p=P),
                    in_=z_w[:].rearrange("p (c e) -> p c e", e=D))
                nc.gpsimd.collective_compute(
                    "AllGather", OP.bypass,
                    replica_groups=[list(range(NCORES))],
                    ins=[cc_z[w][:]], outs=[z_t[w][:]])
                t0 += wt

            # ---- phase A ---------------------------------------------------
            sum_ps = psS.tile([1, P], f32, tag="sum")
            sq_ps = psS.tile([1, P], f32, tag="sq")

            for g in range(ngroups):
                idx_g = metap.tile([P, gcols_i], i16, tag="idxg")
                nc.sync.dma_start(
                    out=idx_g[:], in_=idxs[:, g * gcols_i:(g + 1) * gcols_i])
                eloc_g = metap.tile([P, gcols_e], bf16, tag="elocg")
                nc.sync.dma_start(
                    out=eloc_g[:], in_=eloc[:, g * gcols_e:(g + 1) * gcols_e])

                e_ws = []
                s_ws = []
                ico = 0
                eco = 0
                for w in range(nwin):
                    kw = kws[w]
                    nch = GROUP * kw
                    nidx = nch * P
                    pool = gathA if w < 2 else gathB
                    e_t = pool.tile([P, nch * D], bf16, tag=f"E{w}")
                    nc.gpsimd.dma_gather(
                        e_t[:].rearrange("p (c e) -> p c e", e=D),
                        z_t[w][:],
                        idx_g[:, ico:ico + nidx // 16],
                        nidx, nidx, D, single_packet=False,
                        queue_num=(g + w) % 4)
                    ico += nidx // 16
                    e_ws.append(e_t)
                    s_t = sp.tile([P, nch * P], bf16, tag=f"S{w}")
                    nc.vector.tensor_tensor(
                        out=s_t[:].rearrange("p (c e) -> p c e", e=P),
                        in0=eloc_g[:, eco:eco + nch, None].to_broadcast(
                            [P, nch, P]),
                        in1=iota_rep[:, :nch * P].rearrange(
                            "p (c e) -> p c e", e=P),
                        op=OP.is_equal)
                    eco += nch
                    s_ws.append(s_t)

                h_g = workp.tile([P, GROUP * D], bf16, tag="hg")
                for u in range(GROUP):
                    t = g * GROUP + u
                    agg_ps = psA.tile([P, P], f32, tag="A")
                    ci = 0
                    for w in range(nwin):
                        kw = kws[w]
                        for j in range(kw):
                            nc.tensor.matmul(
                                out=agg_ps[:],
                                lhsT=s_ws[w][:, (u * kw + j) * P:
                                             (u * kw + j + 1) * P],
                                rhs=e_ws[w][:, (u * kw + j) * D:
                                            (u * kw + j + 1) * D],
                                start=(ci + j == 0),
                                stop=(ci + j == ktot - 1))
                        ci += kw
                    h_t = h_g[:, u * D:(u + 1) * D]
                    nc.scalar.activation(out=h_t, in_=agg_ps[:],
                                         func=AF.Copy,
                                         scale=nd_f[:, t:t + 1])
                    sq_sb = workp.tile([P, D], bf16, tag="sqsb")
                    nc.scalar.activation(out=sq_sb[:], in_=h_t,
                                         func=AF.Square)
                    nc.tensor.matmul(out=sum_ps[:], lhsT=ones_c[:], rhs=h_t,
                                     start=(t == 0), stop=(t == nt - 1))
                    nc.tensor.matmul(out=sq_ps[:], lhsT=ones_c[:],
                                     rhs=sq_sb[:],
                                     start=(t == 0), stop=(t == nt - 1))
                nc.scalar.dma_start(
                    out=h_d[:].rearrange("(c p) e -> p c e", p=P)[
                        :, g * GROUP:(g + 1) * GROUP, :],
                    in_=h_g[:].rearrange("p (c e) -> p c e", e=D))

            # ---- BN stats all-reduce + scale/shift (row layout) ------------
            srow = statp.tile([1, 2 * P], f32, tag="srow")
            nc.scalar.copy(srow[0:1, 0:P], sum_ps[:])
            nc.scalar.copy(srow[0:1, P:2 * P], sq_ps[:])
            cc_in = dramp.tile([1, 2 * P], f32, tag="ccin")
            cc_out = dramp.tile([1, 2 * P], f32, tag="ccout")
            nc.gpsimd.dma_start(out=cc_in[:], in_=srow[:])
            nc.gpsimd.collective_compute(
                "AllReduce", OP.add,
                replica_groups=[list(range(NCORES))],
                ins=[cc_in.opt()], outs=[cc_out.opt()])
            grow_sb = statp.tile([1, 2 * P], f32, tag="grow_sb")
            nc.gpsimd.dma_start(out=grow_sb[:], in_=cc_out[:])

            inv_n = 1.0 / float(n_real)
            mean_r = statp.tile([1, P], f32, tag="mean")
            nc.vector.tensor_scalar_mul(mean_r[:], grow_sb[0:1, 0:P], inv_n)
            ex2_r = statp.tile([1, P], f32, tag="ex2")
            nc.vector.tensor_scalar_mul(ex2_r[:], grow_sb[0:1, P:2 * P],
                                        inv_n)
            m2_r = statp.tile([1, P], f32, tag="m2")
            nc.scalar.square(m2_r[:], mean_r[:])
            var_r = statp.tile([1, P], f32, tag="var")
            nc.vector.tensor_tensor(out=var_r[:], in0=ex2_r[:], in1=m2_r[:],
                                    op=OP.subtract)
            nc.vector.tensor_scalar_add(var_r[:], var_r[:], BN_EPS)
            sd_r = statp.tile([1, P], f32, tag="sd")
            nc.scalar.sqrt(sd_r[:], var_r[:])
            inv_r = statp.tile([1, P], f32, tag="inv")
            nc.vector.reciprocal(inv_r[:], sd_r[:])
            sc_r = statp.tile([1, P], f32, tag="sc")
            nc.vector.tensor_tensor(out=sc_r[:], in0=g_row[:], in1=inv_r[:],
                                    op=OP.mult)
            # b cancels in BN: shift = beta - mean*scale
            tc_r = statp.tile([1, P], f32, tag="tc")
            nc.vector.tensor_tensor(out=tc_r[:], in0=mean_r[:], in1=sc_r[:],
                                    op=OP.mult)
            nc.vector.tensor_tensor(out=tc_r[:], in0=be_row[:], in1=tc_r[:],
                                    op=OP.subtract)

            # rank-1 broadcast of sc/tc across partitions -> bf16 tiles
            scb_ps = psA.tile([P, P], f32, tag="A")
            nc.tensor.matmul(out=scb_ps[:], lhsT=ones1[:], rhs=sc_r[:],
                             start=True, stop=True)
            sc_bc = constp.tile([P, P], bf16, tag="sc_bc")
            nc.scalar.copy(sc_bc[:], scb_ps[:])
            tcb_ps = psB.tile([P, P], f32, tag="B")
            nc.tensor.matmul(out=tcb_ps[:], lhsT=ones1[:], rhs=tc_r[:],
                             start=True, stop=True)
            tc_bc = constp.tile([P, P], bf16, tag="tc_bc")
            nc.scalar.copy(tc_bc[:], tcb_ps[:])

            # ---- phase B (batched loads/stores, no transposes) -------------
            for bt in range(nt // PBC):
                t0 = bt * PBC
                qa = nc.scalar if bt % 2 == 0 else nc.sync
                qb = nc.sync if bt % 2 == 0 else nc.scalar
                x_b = iop.tile([P, PBC * D], f32, tag="xb")
                qa.dma_start(
                    out=x_b[:].rearrange("p (c e) -> p c e", e=D),
                    in_=xres[:].rearrange("(c p) e -> p c e", p=P)[
                        :, t0:t0 + PBC, :])
                h_b = iop.tile([P, PBC * D], bf16, tag="hb")
                qb.dma_start(
                    out=h_b[:].rearrange("p (c e) -> p c e", e=D),
                    in_=h_d[:].rearrange("(c p) e -> p c e", p=P)[
                        :, t0:t0 + PBC, :])
                g1 = workp.tile([P, PBC * D], bf16, tag="g1")
                nc.vector.tensor_tensor(
                    out=g1[:].rearrange("p (c e) -> p c e", e=D),
                    in0=h_b[:].rearrange("p (c e) -> p c e", e=D),
                    in1=sc_bc[:, None, :].to_broadcast([P, PBC, D]),
                    op=OP.mult)
                nc.vector.tensor_tensor(
                    out=g1[:].rearrange("p (c e) -> p c e", e=D),
                    in0=g1[:].rearrange("p (c e) -> p c e", e=D),
                    in1=tc_bc[:, None, :].to_broadcast([P, PBC, D]),
                    op=OP.add)
                nc.vector.scalar_tensor_tensor(
                    out=x_b[:], in0=g1[:], scalar=0.0, in1=x_b[:],
                    op0=OP.max, op1=OP.add)
                qb.dma_start(
                    out=out[:].rearrange("(c p) e -> p c e", p=P)[
                        :, t0:t0 + PBC, :],
                    in_=x_b[:].rearrange("p (c e) -> p c e", e=D))

    nc.compile()
    return nc


def _wrap16(a):
    b = a.reshape(-1, 16).T
    return np.tile(b, (8, 1))


def host_prep(x, src, dst, W, b, gamma, beta):
    """Graph routing / layout prep (indices only - no FLOPs on host)."""
    import ml_dtypes

    x = np.asarray(x, np.float32)
    W = np.asarray(W, np.float32)
    gamma = np.asarray(gamma, np.float32)
    beta = np.asarray(beta, np.float32)
    src32 = np.asarray(src).astype(np.int64)
    dst32 = np.asarray(dst).astype(np.int64)

    n = x.shape[0]
    npad = -(-n // (P * NCORES * GROUP)) * (P * NCORES * GROUP)
    nodes_pc = npad // NCORES
    nt = nodes_pc // P
    nt_tot = npad // P
    assert sum(WTILES) == nt
    nwin = len(WTILES)
    wt_start = np.cumsum([0] + list(WTILES))  # in tiles, per core

    order = np.argsort(dst32, kind="stable")
    ds = dst32[order]
    ss = src32[order]

    ar = np.arange(npad + 1, dtype=np.int64)
    rps = np.searchsorted(np.sort(src32), ar).astype(np.int32)
    rpd_full = np.searchsorted(ds, ar)

    # src -> (window, row within window table). Window w of the z table is
    # [core0 quarter w | core1 quarter w | ...], quarter w = tiles
    # [wt_start[w], wt_start[w+1]) of each core's slice.
    s_core = ss // nodes_pc
    s_r = ss % nodes_pc
    s_tile = s_r // P
    s_win = np.searchsorted(wt_start, s_tile, side="right") - 1
    wrows = (np.array(WTILES) * P)[s_win]
    s_winrow = s_core * wrows + (s_r - wt_start[s_win] * P)

    # degree counts (int), F-order [P, nt] per core
    dgo_n = np.diff(rps).astype(np.int32)                 # [npad]
    dgi_n = np.diff(rpd_full).astype(np.int32)            # [npad]

    # per-dst in-degree split by src window
    deg4 = np.zeros((npad, nwin), np.int32)
    np.add.at(deg4, (ds, s_win), 1)

    # --- bin-pack dst nodes into tiles (per core) to flatten the
    # per-(tile, window) edge-count tails, so kws (chunk counts) shrink.
    caps = np.array([4 * P, 5 * P, 5 * P, 4 * P], np.float64)
    newpos = np.empty(npad, np.int64)    # global node -> permuted local slot
    for c in range(NCORES):
        d4 = deg4[c * nodes_pc:(c + 1) * nodes_pc].astype(np.float64)
        order_c = np.argsort(-d4.sum(1), kind="stable")
        loads = np.zeros((nt, nwin))
        counts = np.zeros(nt, np.int64)
        fill = [[] for _ in range(nt)]
        for i in order_c:
            util = np.max((loads + d4[i]) / caps, axis=1)
            util[counts >= P] = np.inf
            b = int(np.argmin(util))
            loads[b] += d4[i]
            counts[b] += 1
            fill[b].append(i)
        pos = np.empty(nodes_pc, np.int64)
        for b in range(nt):
            pos[np.array(fill[b], np.int64)] = (
                b * P + np.arange(len(fill[b])))
        newpos[c * nodes_pc:(c + 1) * nodes_pc] = pos

    # kws from the packed loads (global max over cores/tiles per window)
    e_core = ds // nodes_pc
    e_pos = newpos[ds]
    e_tile = e_core * nt + e_pos // P
    e_slot = e_pos % P
    cell = e_tile * nwin + s_win
    cnt = np.bincount(cell, minlength=nt_tot * nwin).reshape(nt_tot, nwin)
    kws = tuple(int(-(-cnt[:, w].max() // P)) for w in range(nwin))
    ktot = sum(kws)

    # per (tile, window) edge lists under the permutation
    eorder = np.argsort(cell, kind="stable")
    bnd = np.searchsorted(cell[eorder], np.arange(nt_tot * nwin + 1))
    tw_lists = [[eorder[bnd[t * nwin + w]:bnd[t * nwin + w + 1]]
                 for w in range(nwin)] for t in range(nt_tot)]

    xpad = np.zeros((npad, D), np.float32)
    xpad[:n] = x

    ngroups = nt // GROUP
    in_maps = []
    shared = dict(
        wmat=W.astype(ml_dtypes.bfloat16),
        grow=np.ascontiguousarray(gamma[None, :]),
        brow=np.ascontiguousarray(beta[None, :]))
    for c in range(NCORES):
        # eloc layout: (g, w, u, chunk) contiguous for batched expansions
        elocv = np.full((nt * ktot, P), -1.0, np.float32)
        idx_blocks = []
        ecol_off = 0
        for g in range(ngroups):
            for w in range(nwin):
                blk = np.zeros(GROUP * kws[w] * P, np.int16)
                for u in range(GROUP):
                    t = g * GROUP + u
                    gt = c * nt + t
                    sel = tw_lists[gt][w]
                    base = u * kws[w] * P
                    blk[base:base + len(sel)] = s_winrow[sel].astype(np.int16)
                    ev = e_slot[sel].astype(np.float32)
                    ecol = elocv[ecol_off + u * kws[w]:
                                 ecol_off + (u + 1) * kws[w]].reshape(-1)
                    ecol[:len(sel)] = ev
                ecol_off += GROUP * kws[w]
                idx_blocks.append(_wrap16(blk))
        m = dict(shared)
        xslice = xpad[c * nodes_pc:(c + 1) * nodes_pc]
        pos_c = newpos[c * nodes_pc:(c + 1) * nodes_pc]
        orig_of = np.empty(nodes_pc, np.int64)
        orig_of[pos_c] = np.arange(nodes_pc)
        m["xt"] = np.ascontiguousarray(xslice.T).astype(ml_dtypes.bfloat16)
        m["xres"] = np.ascontiguousarray(xslice[orig_of])
        m["dgo"] = np.ascontiguousarray(
            dgo_n[c * nodes_pc:(c + 1) * nodes_pc].reshape(nt, P).T)
        m["dgi"] = np.ascontiguousarray(
            dgi_n[c * nodes_pc:(c + 1) * nodes_pc][orig_of]
            .reshape(nt, P).T)
        m["idxs"] = np.ascontiguousarray(np.concatenate(idx_blocks, axis=1))
        # eloc device layout: col (g,w,u,chunk) partition p = edge c*128+p
        m["eloc"] = np.ascontiguousarray(
            elocv.T).astype(ml_dtypes.bfloat16)
        in_maps.append(m)
    return dict(npad=npad, nt=nt, kws=kws, n_real=n, newpos=newpos), in_maps


def run(in_maps, cfg, **kw):
    from concourse.bass_utils import run_bass_kernel_spmd

    key = (cfg["npad"], cfg["nt"], tuple(cfg["kws"]), cfg["n_real"])
    if key not in _NC_CACHE:
        _NC_CACHE[key] = build_program(*key)
    nc = _NC_CACHE[key]
    res = run_bass_kernel_spmd(nc, in_maps, core_ids=list(range(NCORES)), **kw)
    n = cfg["n_real"]
    nodes_pc = cfg["npad"] // NCORES
    parts = []
    for c in range(NCORES):
        o = np.asarray(res.results[c]["out"])
        pos_c = cfg["newpos"][c * nodes_pc:(c + 1) * nodes_pc]
        parts.append(o[pos_c])
    full = np.concatenate(parts, axis=0)[:n]
    return np.ascontiguousarray(full, dtype=np.float32), res


def kernel(x, src, dst, W, b, gamma, beta):
    cfg, in_maps = host_prep(x, src, dst, W, b, gamma, beta)
    out, _ = run(in_maps, cfg)
    return out
